# revision 1
# baseline (speedup 1.0000x reference)
"""Trainium2 Bass kernel for the ETD1 ODE block (nn_ODEblockW_28922309771809).

Math (mirrors the jax reference, but solve-free):
  s    = 0.05 * sigmoid(alpha)                       # row scales (0.5*dt)
  X    = dt*A = diag(s) @ (adj - I)                  # [2048,2048], ||X|| ~ 0.073
  m1_L = e^X     via degree-8 Taylor, Paterson-Stockmeyer with Y = X^3
  m2   = A^{-1}(e^X - I) = dt*phi1(X),  phi1 = sum_k X^k/(k+1)!   (degree-8 PS)
  B    = (w*clip(d,0,1)) @ w.T - I  (symmetric);  Xr = dt*B;  m1_R = e^{Xr}
  F    = m2 @ x0
  z    = IC after 9 steps of IC <- m1_L @ IC @ m1_R + F   (N_STEPS = int(1.0//0.1) == 9)

Distribution over 8 cores (transposed-column-local formulation):
  The node dim (2048) is sharded 256 rows/core; every local tensor is held as
  the transposed column block [2048|1024, 256], so each big matmul is
     out_colT[m] = sum_k  matmul(lhsT = Full[kblk, mblk] from DRAM, rhs = colT[kblk])
  Full matrices are assembled by AllGather of row blocks (PE-transpose of the
  local column block first). The feature dim (1024) is sharded 128/core.

  AllGathered tensors use a TILED layout: each rank's contribution is a
  sequence of [128,128] tiles (m-major), so the per-m lhsT slab loads read
  8-16 contiguous 32-64KB blocks instead of 256B-strided rows. Node-dim
  gathers are split into two pipelined half-gathers (half j carries k-chunks
  k%2==j); consuming matmuls run even k-chunks first so they start as soon as
  the first half lands.

Precision: series matmuls in bf16 (bf16 error only enters quadratic+ Taylor
terms of e^X; the I and X terms are exact fp32 elementwise), recurrence /
forcing / R-side matmuls in float32r. Measured ~9.5e-4 frob rel err vs the
fp32 reference, which itself carries ~1.7e-4 fp32 rounding noise vs fp64.
"""

import math
from contextlib import ExitStack

import numpy as np

import concourse.bass as bass
import concourse.mybir as mybir
import concourse.tile as tile
from concourse import bacc
from concourse.bass_utils import run_bass_kernel_spmd
from concourse.masks import make_identity

F32 = mybir.dt.float32
F32R = mybir.dt.float32r
BF16 = mybir.dt.bfloat16
AL = mybir.AluOpType

N_CORES = 8
P = 128
N = 2048          # nodes
D = 1024          # features
RB = 256          # node row-block per core
FB = 256          # node col-block width (L side)
FBR = 128         # feature block width (R side, true 8-way shard)
NKC = N // P      # 16
DKC = D // P      # 8
RJ = RB // P      # 2
NSTEPS = 9        # int(1.0 // 0.1) == 9

EC = [1.0 / math.factorial(k) for k in range(9)]        # e^X coeffs
PC = [0.1 / math.factorial(k + 1) for k in range(9)]    # dt*phi1(X) coeffs

LGROUP = [list(range(N_CORES))]


def build_nc():
    nc = bacc.Bacc("TRN2", target_bir_lowering=False, debug=False,
                   num_devices=N_CORES)

    # ---- I/O (per-core shards fed host-side; same NEFF on all cores) ----
    adj_rows = nc.dram_tensor("adj_rows", [RB, N], F32, kind="ExternalInput")
    eye_rows = nc.dram_tensor("eye_rows", [RB, N], F32, kind="ExternalInput")
    eye_colT = nc.dram_tensor("eye_colT", [N, RB], F32, kind="ExternalInput")
    alpha_blk = nc.dram_tensor("alpha_blk", [RB], F32, kind="ExternalInput")
    x_full = nc.dram_tensor("x_full", [N, D], F32, kind="ExternalInput")
    x0_full = nc.dram_tensor("x0_full", [N, D], F32, kind="ExternalInput")
    w_cols = nc.dram_tensor("w_cols", [D, FBR], F32, kind="ExternalInput")
    w_rows = nc.dram_tensor("w_rows", [FBR, D], F32, kind="ExternalInput")
    eye_feat = nc.dram_tensor("eye_feat", [D, FBR], F32, kind="ExternalInput")
    d_full = nc.dram_tensor("d_full", [D], F32, kind="ExternalInput")
    z_loc = nc.dram_tensor("z_loc", [RB, D], F32, kind="ExternalOutput")

    with tile.TileContext(nc) as tc, ExitStack() as top:
        const = top.enter_context(tc.tile_pool(name="const", bufs=1))
        dram = top.enter_context(tc.tile_pool(name="dram", bufs=1, space="DRAM"))
        psum = top.enter_context(tc.tile_pool(name="psum", bufs=2, space="PSUM"))
        slabp = top.enter_context(tc.tile_pool(name="slabp", bufs=1))
        scrp = top.enter_context(tc.tile_pool(name="scrp", bufs=1))
        lser = top.enter_context(tc.tile_pool(name="lser", bufs=1))
        lout = top.enter_context(tc.tile_pool(name="lout", bufs=1))

        ident = const.tile([P, P], F32)
        make_identity(nc, ident)
        ident_b = const.tile([P, P], BF16)
        nc.vector.tensor_copy(ident_b[:], ident[:])

        def pe_t(dst_slice, src_slice, bf=False):
            """dst[128,128] = src[128,128].T via PE transpose."""
            if src_slice.dtype == F32R:
                src_slice = src_slice.bitcast(F32)
            ps = psum.tile([P, P], BF16 if bf else F32, tag="tr", bufs=4, name="ps_tr")
            nc.tensor.transpose(ps[:], src_slice, ident_b[:] if bf else ident[:])
            nc.vector.tensor_copy(dst_slice, ps[:])

        def combo(dst_slice, eye_m, xt_m, x2t_m, c0, c1, c2):
            """dst = c0*I + c1*X + c2*X2 for one [128,w] chunk."""
            if xt_m.dtype == F32R:
                xt_m = xt_m.bitcast(F32)
            if x2t_m.dtype == F32R:
                x2t_m = x2t_m.bitcast(F32)
            w = xt_m.shape[-1]
            st = scrp.tile([P, FB], F32, tag="combo", bufs=3, name="combo_scr")
            s = st[:, :w]
            nc.vector.tensor_scalar_mul(s, xt_m, c1)
            nc.vector.scalar_tensor_tensor(s, x2t_m, c2, s, AL.mult, AL.add)
            nc.vector.scalar_tensor_tensor(dst_slice, eye_m, c0, s, AL.mult, AL.add)

        def load_eye(dram_t, m, w=FB):
            t = scrp.tile([P, FB], F32, tag="eye", bufs=2, name="eye_chunk")
            nc.sync.dma_start(t[:, :w], dram_t[m * P:(m + 1) * P, :])
            return t[:, :w]

        def _bc(src_ap, dt):
            if dt == F32R and src_ap.dtype == F32:
                return src_ap.bitcast(F32R)
            return src_ap

        # ---- tiled-gather helpers -------------------------------------
        # A gathered tensor is [ranks * tiles * P, P]: rank c's contribution
        # is `tiles` contiguous [128,128] tiles (tile t = cols t*128 of the
        # rank's [128, tiles*128] row block).
        def put_tiles(ccin, row_sb, tiles):
            """DMA row block row_sb [128, tiles*128] into tiled ccin."""
            for t in range(tiles):
                nc.sync.dma_start(ccin[t * P:(t + 1) * P, :],
                                  row_sb[:, t * P:(t + 1) * P])

        def tiled_src(g, m, jpr, tiles, dt):
            """AP over gathered g: [128, ranks, jpr, 128] = tile m of every
            rank's jpr row-chunks. Contribution tile order: j-major, m-minor."""
            a = _bc(g[:], dt).rearrange("(c j t p) n -> p c j t n", c=N_CORES,
                                        j=jpr, t=tiles, p=P)
            return a[:, :, :, m, :]

        def gather_tiled(produce, jpr, tiles, dt, name):
            """Single AllGather with tiled contribution: jpr row-chunks of
            `tiles` [128,128] tiles each. produce(ccin) fills it."""
            ccin = dram.tile([jpr * tiles * P, P], dt, tag=f"ccin_{name}",
                             name=f"ccin_{name}")
            full = dram.tile([N_CORES * jpr * tiles * P, P], dt,
                             addr_space="Shared", name=f"full_{name}")
            produce(ccin)
            nc.gpsimd.collective_compute(
                "AllGather", AL.bypass, replica_groups=LGROUP,
                ins=[ccin.opt()], outs=[full.opt()])
            return full

        def mm_pass(rhs_tiles, n_k, n_m, evict, dt, tag, nb=FB,
                    g=None, jpr=1, plain=None, tiles=None):
            """For each output chunk m: psums[i] = sum_k lhsT[k,m].T @ rhs[i][k].

            lhsT source: either `plain` (a [n_k*P, n_m*P] DRAM AP, k-chunk k at
            rows k*128) or `g` (a tiled-gathered tensor where k-chunk k lives
            as rank k//jpr, row-chunk k%jpr). For dt == F32R the rhs tiles
            must already be float32r-dtyped."""
            tiles_ = n_m if tiles is None else tiles
            for m in range(n_m):
                if g is not None:
                    sl = slabp.tile([P, N_CORES, jpr, P], dt, tag=tag,
                                    bufs=2, name=f"slab_{tag}")
                    nc.sync.dma_start(sl[:], tiled_src(g, m, jpr, tiles_, dt))
                    lt = lambda k: sl[:, k // jpr, k % jpr, :]
                else:
                    sl = slabp.tile([P, n_k, P], dt, tag=tag, bufs=2,
                                    name=f"slab_{tag}")
                    src = _bc(plain[:, m * P:(m + 1) * P], dt)
                    nc.sync.dma_start(sl[:], src.rearrange("(k p) n -> p k n", p=P))
                    lt = lambda k: sl[:, k, :]
                pss = [psum.tile([P, nb], F32, tag=f"mm{i}", bufs=2,
                                 name=f"ps_mm{i}") for i in range(len(rhs_tiles))]
                for k in range(n_k):
                    for ps, rhs in zip(pss, rhs_tiles):
                        nc.tensor.matmul(ps[:], lt(k), rhs[:, k, :],
                                         start=(k == 0), stop=(k == n_k - 1))
                evict(m, pss)

        # =========================================================
        # Prep scales
        # =========================================================
        s_sb = const.tile([P, RJ], F32)
        nc.sync.dma_start(s_sb[:], alpha_blk.ap().rearrange("(j p) -> p j", p=P))
        nc.scalar.activation(s_sb[:], s_sb[:], mybir.ActivationFunctionType.Sigmoid)
        nc.vector.tensor_scalar_mul(s_sb[:], s_sb[:], 0.05)

        d_sb = const.tile([P, DKC], F32)
        nc.sync.dma_start(d_sb[:], d_full.ap().rearrange("(q p) -> p q", p=P))
        nc.vector.tensor_scalar(d_sb[:], d_sb[:], 0.0, 1.0, AL.max, AL.min)

        xt = lser.tile([P, NKC, FB], F32)     # X^T col block, fp32
        x2t = lser.tile([P, NKC, FB], F32)    # (X^2)^T col block, fp32
        et = lout.tile([P, NKC, FB], F32R)    # m1_L^T col block
        m2t = lout.tile([P, NKC, FB], F32R)   # m2^T col block

        # =========================================================
        # Emission order interleaves the R-side (feature dim) chain between
        # the L-side passes: engine queues are in-order, so each R compute
        # segment is emitted one L-pass after the gather it depends on —
        # its semaphore wait is satisfied by the time the PE reaches it.
        # =========================================================
        pa_st, pr_st = ExitStack(), ExitStack()
        pr = pr_st.enter_context(tc.tile_pool(name="ph_r", bufs=1))
        pa = pa_st.enter_context(tc.tile_pool(name="ph_a", bufs=1))

        # --- R prep: w^T row block -> AllGather (earliest collective) ---
        wt_rowblk = pr.tile([P, D], F32)
        for k in range(DKC):
            wc_sb = pr.tile([P, FBR], F32, tag="w_in", bufs=2, name="wc_sb")
            nc.sync.dma_start(wc_sb[:], w_cols[k * P:(k + 1) * P, :])
            pe_t(wt_rowblk[:, k * P:(k + 1) * P], wc_sb[:])
        wt_g = gather_tiled(lambda ccin: put_tiles(ccin, wt_rowblk[:], DKC),
                            1, DKC, F32, "wt")

        # V = diag(clip(d)) @ w^T[:, Fblk]   [1024, 128]
        vr = pr.tile([P, DKC, FBR], F32R)
        wr_sb = pr.tile([P, D], F32, name="wr_sb")
        nc.sync.dma_start(wr_sb[:], w_rows[:])
        for k in range(DKC):
            pe_t(vr[:, k, :], wr_sb[:, k * P:(k + 1) * P])
        for k in range(DKC):
            nc.vector.tensor_scalar_mul(vr[:, k, :], vr[:, k, :].bitcast(F32),
                                        d_sb[:, k:k + 1])

        # --- Phase A: build X row block, AllGather X (bf16), transpose ---
        xrow = pa.tile([P, RJ, N], F32)
        xrow_b = pa.tile([P, RJ, N], BF16)
        ccin_x = dram.tile([RJ * NKC * P, P], BF16, name="ccin_x")
        for j in range(RJ):
            adj_sb = pa.tile([P, N], F32, tag="a_in", bufs=2, name="adj_sb")
            eyer_sb = pa.tile([P, N], F32, tag="a_in", bufs=2, name="eyer_sb")
            nc.sync.dma_start(adj_sb[:], adj_rows[j * P:(j + 1) * P, :])
            nc.sync.dma_start(eyer_sb[:], eye_rows[j * P:(j + 1) * P, :])
            nc.vector.tensor_sub(adj_sb[:], adj_sb[:], eyer_sb[:])
            nc.vector.tensor_scalar_mul(xrow[:, j, :], adj_sb[:], s_sb[:, j:j + 1])
            nc.vector.tensor_copy(xrow_b[:, j, :], xrow[:, j, :])
            put_tiles(ccin_x[j * NKC * P:(j + 1) * NKC * P, :], xrow_b[:, j, :], NKC)
        xfull_g = dram.tile([N_CORES * RJ * NKC * P, P], BF16,
                            addr_space="Shared", name="full_x")
        nc.gpsimd.collective_compute(
            "AllGather", AL.bypass, replica_groups=LGROUP,
            ins=[ccin_x.opt()], outs=[xfull_g.opt()])

        for k in range(NKC):
            for j in range(RJ):
                pe_t(xt[:, k, j * P:(j + 1) * P], xrow[:, j, k * P:(k + 1) * P])
        pa_st.close()

        # --- R: w_mat col block -> Xr = 0.1*(w_mat - I); gather Xr ---
        xr_col = pr.tile([P, DKC, FBR], F32R)

        def ev_wmat(m, pss):
            eyef = load_eye(eye_feat, m, FBR)
            nc.vector.tensor_sub(xr_col[:, m, :], pss[0][:], eyef)
            nc.vector.tensor_scalar_mul(xr_col[:, m, :],
                                        xr_col[:, m, :].bitcast(F32), 0.1)
        mm_pass([vr], DKC, DKC, ev_wmat, F32R, "fslab", nb=FBR,
                g=wt_g, tiles=DKC)

        def gather_sym(col_tile, name):
            """Symmetric [D,D] matrix: transpose col block -> row block -> AG."""
            rowblk = pr.tile([P, D], F32, tag="r_rowblk", bufs=2,
                             name=f"rowblk_{name}")
            for k in range(DKC):
                pe_t(rowblk[:, k * P:(k + 1) * P], col_tile[:, k, :])
            return gather_tiled(lambda ccin: put_tiles(ccin, rowblk[:], DKC),
                                1, DKC, F32, name)

        xr_g = gather_sym(xr_col, "xr")

        # --- Phase C1: X^2 (bf16) ---
        pc_st = ExitStack()
        pc_ = pc_st.enter_context(tc.tile_pool(name="ph_c", bufs=1))
        xt_b = pc_.tile([P, NKC, FB], BF16)
        nc.vector.tensor_copy(xt_b[:], xt[:])
        x2t_b = pc_.tile([P, NKC, FB], BF16)

        def ev_x2(m, pss):
            nc.vector.tensor_copy(x2t[:, m, :], pss[0][:])
            nc.vector.tensor_copy(x2t_b[:, m, :], pss[0][:])
        mm_pass([xt_b], NKC, NKC, ev_x2, BF16, "xslab", g=xfull_g, jpr=RJ)

        # --- R: Xr^2, Xr^3 (gathers hidden under the X^2 pass) ---
        xr2_col = pr.tile([P, DKC, FBR], F32R)
        mm_pass([xr_col], DKC, DKC,
                lambda m, pss: nc.vector.tensor_copy(xr2_col[:, m, :], pss[0][:]),
                F32R, "fslab", nb=FBR, g=xr_g, tiles=DKC)
        xr3_col = pr.tile([P, DKC, FBR], F32)
        mm_pass([xr2_col], DKC, DKC,
                lambda m, pss: nc.vector.tensor_copy(xr3_col[:, m, :], pss[0][:]),
                F32R, "fslab", nb=FBR, g=xr_g, tiles=DKC)
        xr3_g = gather_sym(xr3_col, "xr3")

        # --- Phase C2: X^3 (bf16) ---
        x3t_b = pc_.tile([P, NKC, FB], BF16)
        mm_pass([x2t_b], NKC, NKC,
                lambda m, pss: nc.vector.tensor_copy(x3t_b[:, m, :], pss[0][:]),
                BF16, "xslab", g=xfull_g, jpr=RJ)

        x3row_b = pc_.tile([P, RJ, N], BF16)
        ccin_x3 = dram.tile([RJ * NKC * P, P], BF16, name="ccin_x3")
        for j in range(RJ):
            for k in range(NKC):
                pe_t(x3row_b[:, j, k * P:(k + 1) * P],
                     x3t_b[:, k, j * P:(j + 1) * P], bf=True)
            put_tiles(ccin_x3[j * NKC * P:(j + 1) * NKC * P, :],
                      x3row_b[:, j, :], NKC)
        x3full_g = dram.tile([N_CORES * RJ * NKC * P, P], BF16,
                             addr_space="Shared", name="full_x3")
        nc.gpsimd.collective_compute(
            "AllGather", AL.bypass, replica_groups=LGROUP,
            ins=[ccin_x3.opt()], outs=[x3full_g.opt()])

        pc_st.close()

        # --- R: T_R = B1r + Y*B2r ; m1_R = B0r + Y*T_R  (xr3 gather done
        #     during the X^3 pass) ---
        b2r = pr.tile([P, DKC, FBR], F32R)
        for m in range(DKC):
            eyef = load_eye(eye_feat, m, FBR)
            combo(b2r[:, m, :], eyef, xr_col[:, m, :], xr2_col[:, m, :],
                  EC[6], EC[7], EC[8])
        tr_col = pr.tile([P, DKC, FBR], F32R)

        def ev_tr(m, pss):
            eyef = load_eye(eye_feat, m, FBR)
            b1t = scrp.tile([P, FB], F32, tag="combo", bufs=3, name="b1_scr")
            b1 = b1t[:, :FBR]
            combo(b1, eyef, xr_col[:, m, :], xr2_col[:, m, :],
                  EC[3], EC[4], EC[5])
            nc.vector.tensor_add(tr_col[:, m, :], pss[0][:], b1)
        mm_pass([b2r], DKC, DKC, ev_tr, F32R, "fslab", nb=FBR,
                g=xr3_g, tiles=DKC)

        m1r_col = pr.tile([P, DKC, FBR], F32)

        def ev_m1r(m, pss):
            eyef = load_eye(eye_feat, m, FBR)
            b0t = scrp.tile([P, FB], F32, tag="combo", bufs=3, name="b0_scr")
            b0 = b0t[:, :FBR]
            combo(b0, eyef, xr_col[:, m, :], xr2_col[:, m, :],
                  EC[0], EC[1], EC[2])
            nc.vector.tensor_add(m1r_col[:, m, :], pss[0][:], b0)
        mm_pass([tr_col], DKC, DKC, ev_m1r, F32R, "fslab", nb=FBR,
                g=xr3_g, tiles=DKC)

        m1r_g = gather_sym(m1r_col, "m1r")
        pr_st.close()

        # --- Phase D: T/S then E/P Horner steps (bf16) ---
        pd_st = ExitStack()
        pd = pd_st.enter_context(tc.tile_pool(name="ph_d", bufs=1))
        b2e_b = pd.tile([P, NKC, FB], BF16)
        c2p_b = pd.tile([P, NKC, FB], BF16)
        for m in range(NKC):
            eyet = load_eye(eye_colT, m)
            combo(b2e_b[:, m, :], eyet, xt[:, m, :], x2t[:, m, :],
                  EC[6], EC[7], EC[8])
            combo(c2p_b[:, m, :], eyet, xt[:, m, :], x2t[:, m, :],
                  PC[6], PC[7], PC[8])

        tt_b = pd.tile([P, NKC, FB], BF16)
        st_b = pd.tile([P, NKC, FB], BF16)

        def ev_ts(m, pss):
            eyet = load_eye(eye_colT, m)
            b1 = scrp.tile([P, FB], F32, tag="combo", bufs=3, name="ts_scr")
            combo(b1[:], eyet, xt[:, m, :], x2t[:, m, :], EC[3], EC[4], EC[5])
            nc.vector.tensor_add(tt_b[:, m, :], pss[0][:], b1[:])
            combo(b1[:], eyet, xt[:, m, :], x2t[:, m, :], PC[3], PC[4], PC[5])
            nc.vector.tensor_add(st_b[:, m, :], pss[1][:], b1[:])
        mm_pass([b2e_b, c2p_b], NKC, NKC, ev_ts, BF16, "xslab",
                g=x3full_g, jpr=RJ)

        def ev_ep(m, pss):
            eyet = load_eye(eye_colT, m)
            b0 = scrp.tile([P, FB], F32, tag="combo", bufs=3, name="ep_scr")
            combo(b0[:], eyet, xt[:, m, :], x2t[:, m, :], EC[0], EC[1], EC[2])
            nc.vector.tensor_add(et[:, m, :], pss[0][:], b0[:])
            combo(b0[:], eyet, xt[:, m, :], x2t[:, m, :], PC[0], PC[1], PC[2])
            nc.vector.tensor_add(m2t[:, m, :], pss[1][:], b0[:])
        mm_pass([tt_b, st_b], NKC, NKC, ev_ep, BF16, "xslab",
                g=x3full_g, jpr=RJ)

        pd_st.close()

        # --- Phase E: forcing + 9-step recurrence (fp32r) ---
        pe = top.enter_context(tc.tile_pool(name="ph_e", bufs=1))
        m1r_sb = pe.tile([P, DKC, DKC, P], F32R)
        nc.sync.dma_start(
            m1r_sb[:],
            m1r_g[:].bitcast(F32R).rearrange("(c t p) n -> p c t n",
                                             c=N_CORES, t=DKC, p=P))

        ft = pe.tile([P, DKC, FB], F32)
        mm_pass([m2t], NKC, DKC,
                lambda m, pss: nc.vector.tensor_copy(ft[:, m, :], pss[0][:]),
                F32R, "icslab0", plain=x0_full[:])

        ic_g = None
        for t in range(NSTEPS):
            # V = (m1_L @ IC)^T col block = IC^T-contract with m1_L^T col
            v = pe.tile([P, DKC, FB], F32R, tag="v_step", bufs=2, name="v")
            if t == 0:
                mm_pass([et], NKC, DKC,
                        lambda m, pss: nc.vector.tensor_copy(v[:, m, :], pss[0][:]),
                        F32R, "icslab0", plain=x_full[:])
            else:
                mm_pass([et], NKC, DKC,
                        lambda m, pss: nc.vector.tensor_copy(v[:, m, :], pss[0][:]),
                        F32R, "icslab", g=ic_g, jpr=RJ, tiles=DKC)
            # IC_new^T col = m1_R-contract with V + F^T
            icnt = pe.tile([P, DKC, FB], F32, tag="icnt_step", bufs=2, name="icnt")
            for m in range(DKC):
                ps = psum.tile([P, FB], F32, tag="mm0", bufs=2, name="ps_rec")
                for k in range(DKC):
                    nc.tensor.matmul(
                        ps[:], m1r_sb[:, k, m, :], v[:, k, :],
                        start=(k == 0), stop=(k == DKC - 1))
                nc.vector.tensor_add(icnt[:, m, :], ps[:], ft[:, m, :])
            # transpose to row block; DMA tiles out as they complete
            icrow = pe.tile([P, RJ, D], F32, tag="icrow_step", bufs=2, name="icrow")
            if t < NSTEPS - 1:
                ccin_ic = dram.tile([RJ * DKC * P, P], F32, tag="ccin_ic",
                                    name=f"ccin_ic{t}")
                for j in range(RJ):
                    for m in range(DKC):
                        pe_t(icrow[:, j, m * P:(m + 1) * P],
                             icnt[:, m, j * P:(j + 1) * P])
                        nc.sync.dma_start(
                            ccin_ic[(j * DKC + m) * P:(j * DKC + m + 1) * P, :],
                            icrow[:, j, m * P:(m + 1) * P])
                ic_g = dram.tile([N_CORES * RJ * DKC * P, P], F32,
                                 addr_space="Shared", name=f"full_ic{t}")
                nc.gpsimd.collective_compute(
                    "AllGather", AL.bypass, replica_groups=LGROUP,
                    ins=[ccin_ic.opt()], outs=[ic_g.opt()])
            else:
                for j in range(RJ):
                    for m in range(DKC):
                        pe_t(icrow[:, j, m * P:(m + 1) * P],
                             icnt[:, m, j * P:(j + 1) * P])
                    nc.sync.dma_start(z_loc[j * P:(j + 1) * P, :], icrow[:, j, :])

    nc.compile()
    return nc


_NC_CACHE = []


def _get_nc():
    if not _NC_CACHE:
        _NC_CACHE.append(build_nc())
    return _NC_CACHE[0]


def make_in_maps(inputs):
    x = np.ascontiguousarray(np.asarray(inputs["x"], dtype=np.float32))
    x0 = np.ascontiguousarray(np.asarray(inputs["x0"], dtype=np.float32))
    adj = np.ascontiguousarray(np.asarray(inputs["adj"], dtype=np.float32))
    alpha = np.ascontiguousarray(np.asarray(inputs["alpha_train"], dtype=np.float32))
    w = np.ascontiguousarray(np.asarray(inputs["w"], dtype=np.float32))
    d = np.ascontiguousarray(np.asarray(inputs["d"], dtype=np.float32))

    eye_n = np.eye(N, dtype=np.float32)
    eye_d = np.eye(D, dtype=np.float32)

    in_maps = []
    for c in range(N_CORES):
        r0 = c * RB
        f0 = c * FBR
        in_maps.append({
            "adj_rows": np.ascontiguousarray(adj[r0:r0 + RB, :]),
            "eye_rows": np.ascontiguousarray(eye_n[r0:r0 + RB, :]),
            "eye_colT": np.ascontiguousarray(eye_n[:, r0:r0 + RB]),
            "alpha_blk": np.ascontiguousarray(alpha[r0:r0 + RB]),
            "x_full": x,
            "x0_full": x0,
            "w_cols": np.ascontiguousarray(w[:, f0:f0 + FBR]),
            "w_rows": np.ascontiguousarray(w[f0:f0 + FBR, :]),
            "eye_feat": np.ascontiguousarray(eye_d[:, f0:f0 + FBR]),
            "d_full": d,
        })
    return in_maps


def kernel(**inputs) -> np.ndarray:
    nc = _get_nc()
    in_maps = make_in_maps(inputs)
    res = run_bass_kernel_spmd(nc, in_maps, core_ids=list(range(N_CORES)))
    z = np.concatenate([res.results[c]["z_loc"] for c in range(N_CORES)], axis=0)
    return np.ascontiguousarray(z.astype(np.float32))


if __name__ == "__main__":
    rng = np.random.default_rng(0)
    ins = {
        "x": rng.standard_normal((N, D)).astype(np.float32),
        "x0": rng.standard_normal((N, D)).astype(np.float32),
        "adj": (rng.random((N, N)) / N).astype(np.float32),
        "alpha_train": rng.standard_normal((N,)).astype(np.float32),
        "w": (np.eye(D) + 0.02 * rng.standard_normal((D, D))).astype(np.float32),
        "d": rng.random((D,)).astype(np.float32),
    }
    out = kernel(**ins)
    print("kernel output:", out.shape, out.dtype, float(np.linalg.norm(out)))



# revision 10
# speedup vs baseline: 1.7874x; 1.7874x over previous
"""Trainium2 Bass kernel for the ETD1 ODE block (nn_ODEblockW_28922309771809).

Math (mirrors the jax reference; degree-4 Taylor is exact to ~1e-7 here
since ||dt*A||_2 ~ 0.05 and ||dt*B||_2 ~ 0.16):
  s    = 0.05 * sigmoid(alpha)                      # row scales (0.5*dt)
  X    = diag(s) @ (adj - I)          [2048x2048]
  Y    = X^2
  E_L  = m1_L - I = X + Y(I/2 + X/6 + Y/24)         # deg-4 e^X, PS in Y
  m2   = dt*phi1(X) = 0.1(I + X/2) + Y(0.1)(I/6 + X/24 + Y/120)
  Xr   = 0.1((w*clip(d,0,1)) @ w.T - I);  Yr = Xr^2
  E_R  = m1_R - I = Xr + Yr(I/2 + Xr/6 + Yr/24)
  F    = m2 @ x0
  9 steps of: V = IC + E_L@IC ;  IC <- V + V@E_R + F

Distribution over 8 cores (transposed-column-local formulation): node dim
sharded 256 rows/core, feature dim 128/core; local tensors held as the
transposed column block [2048|1024, 256|128].  Full matrices needed as
matmul lhsT come either from replicated DRAM inputs (adj-I, w^T, x, x0 —
no collective needed) or from AllGathers of computed tensors (Y, Xr, Yr,
E_R, and IC each step).  All gathered tensors are bf16 in a tiled layout
([128,128] tiles); the Y and per-step IC gathers are split in two halves
so consuming matmul chunks start as soon as the first half lands.

Precision: quadratic-and-higher series terms and all recurrence matmul
inputs in bf16; linear terms and the recurrence state accumulation in
fp32.  Because E_L/E_R are near-zero (not near-identity), bf16 rounding
of matmul inputs is scaled by ||E|| ~ 0.1 per step.  Measured ~1.3e-3
frob rel err vs the fp32 reference (tolerance 2e-2).
"""

from contextlib import ExitStack

import numpy as np
import ml_dtypes

import concourse.bass as bass
import concourse.mybir as mybir
import concourse.tile as tile
from concourse import bacc
from concourse.bass_utils import run_bass_kernel_spmd
from concourse.masks import make_identity

F32 = mybir.dt.float32
F32R = mybir.dt.float32r
BF16 = mybir.dt.bfloat16
AL = mybir.AluOpType

N_CORES = 8
P = 128
N = 2048          # nodes
D = 1024          # features
RB = 256          # node row-block per core
FB = 256          # L-side col width (= RB)
FBR = 128         # feature block width per core
NKC = N // P      # 16
DKC = D // P      # 8
RJ = RB // P      # 2
NSTEPS = 9        # int(1.0 // 0.1)

LGROUP = [list(range(N_CORES))]
BFNP = ml_dtypes.bfloat16


def build_nc():
    nc = bacc.Bacc("TRN2", target_bir_lowering=False, debug=False,
                   num_devices=N_CORES)

    # ---- per-core inputs ----
    adjmi_rows = nc.dram_tensor("adjmi_rows", [RB, N], BF16, kind="ExternalInput")
    alpha_blk = nc.dram_tensor("alpha_blk", [RB], F32, kind="ExternalInput")
    x_colT = nc.dram_tensor("x_colT", [D, RB], F32, kind="ExternalInput")
    w_colT = nc.dram_tensor("w_colT", [D, FBR], F32, kind="ExternalInput")
    masksL = nc.dram_tensor("masksL", [P, NKC * 2], F32, kind="ExternalInput")
    masksR = nc.dram_tensor("masksR", [P, DKC], F32, kind="ExternalInput")
    # ---- replicated inputs (same array on every core) ----
    adjmi_bf = nc.dram_tensor("adjmi_bf", [N, N], BF16, kind="ExternalInput")
    alpha_full = nc.dram_tensor("alpha_full", [N], F32, kind="ExternalInput")
    x_bf = nc.dram_tensor("x_bf", [N, D], BF16, kind="ExternalInput")
    x0_bf = nc.dram_tensor("x0_bf", [N, D], BF16, kind="ExternalInput")
    wT_full = nc.dram_tensor("wT_full", [D, D], F32, kind="ExternalInput")
    d_full = nc.dram_tensor("d_full", [D], F32, kind="ExternalInput")
    z_loc = nc.dram_tensor("z_loc", [RB, D], F32, kind="ExternalOutput")

    with tile.TileContext(nc) as tc, ExitStack() as top:
        const = top.enter_context(tc.tile_pool(name="const", bufs=1))
        dram = top.enter_context(tc.tile_pool(name="dram", bufs=1, space="DRAM"))
        psum = top.enter_context(tc.tile_pool(name="psum", bufs=2, space="PSUM"))
        slabp = top.enter_context(tc.tile_pool(name="slabp", bufs=1))
        scrp = top.enter_context(tc.tile_pool(name="scrp", bufs=1))
        main = top.enter_context(tc.tile_pool(name="main", bufs=1))

        ident = const.tile([P, P], F32)
        make_identity(nc, ident)
        ident_b = const.tile([P, P], BF16)
        nc.vector.tensor_copy(ident_b[:], ident[:])
        # scaled identity tiles for masked diagonal adds
        id_te = const.tile([P, P], BF16)       # 0.5 I
        nc.vector.tensor_scalar_mul(id_te[:], ident[:], 0.5)
        id_tp = const.tile([P, P], BF16)       # (0.1/6) I
        nc.vector.tensor_scalar_mul(id_tp[:], ident[:], 0.1 / 6.0)
        id_m2 = const.tile([P, P], BF16)       # 0.1 I
        nc.vector.tensor_scalar_mul(id_m2[:], ident[:], 0.1)
        idn01 = const.tile([P, P], BF16)       # -0.1 I
        nc.vector.tensor_scalar_mul(idn01[:], ident[:], -0.1)

        mL = const.tile([P, NKC * 2], F32)
        nc.sync.dma_start(mL[:], masksL[:])
        mR = const.tile([P, DKC], F32)
        nc.sync.dma_start(mR[:], masksR[:])

        def pe_t(dst_slice, src_slice):
            """dst[128,128] = src[128,128].T via PE transpose."""
            if src_slice.dtype == F32R:
                src_slice = src_slice.bitcast(F32)
            fp32_in = src_slice.dtype == F32
            ps = psum.tile([P, P], F32 if fp32_in else BF16, tag="tr",
                           bufs=2, name="ps_tr")
            nc.tensor.transpose(ps[:], src_slice,
                                ident[:] if fp32_in else ident_b[:])
            nc.vector.tensor_copy(dst_slice, ps[:])

        # =========================================================
        # scales: s = 0.05*sigmoid(alpha)  (own rows + full), d clip
        # =========================================================
        s_sb = const.tile([P, RJ], F32)
        nc.sync.dma_start(s_sb[:], alpha_blk.ap().rearrange("(j p) -> p j", p=P))
        nc.scalar.activation(s_sb[:], s_sb[:],
                             mybir.ActivationFunctionType.Sigmoid)
        nc.vector.tensor_scalar_mul(s_sb[:], s_sb[:], 0.05)

        s_full = const.tile([P, NKC], F32)
        nc.sync.dma_start(s_full[:], alpha_full.ap().rearrange("(k p) -> p k", p=P))
        nc.scalar.activation(s_full[:], s_full[:],
                             mybir.ActivationFunctionType.Sigmoid)
        nc.vector.tensor_scalar_mul(s_full[:], s_full[:], 0.05)

        d_sb = const.tile([P, DKC], F32)
        nc.sync.dma_start(d_sb[:], d_full.ap().rearrange("(q p) -> p q", p=P))
        nc.vector.tensor_scalar(d_sb[:], d_sb[:], 0.0, 1.0, AL.max, AL.min)

        # =========================================================
        # local col blocks: xt_b = X^T[:, own 256 cols]  (bf16)
        # =========================================================
        ser_st = ExitStack()
        ser = ser_st.enter_context(tc.tile_pool(name="ph_ser", bufs=1))
        m2_st = ExitStack()
        m2p = m2_st.enter_context(tc.tile_pool(name="ph_m2", bufs=1))
        rows_st = ExitStack()
        rowsp = rows_st.enter_context(tc.tile_pool(name="ph_rows", bufs=1))

        xt_b = ser.tile([P, NKC, FB], BF16)
        with tc.tile_pool(name="ph_x", bufs=1) as ph_x:
            xrow_b = ph_x.tile([P, RJ, N], BF16)
            for j in range(RJ):
                raw = scrp.tile([P, N], BF16, tag="adj_in", bufs=1, name="adj_in")
                nc.sync.dma_start(raw[:], adjmi_rows[j * P:(j + 1) * P, :])
                nc.vector.tensor_scalar_mul(xrow_b[:, j, :], raw[:],
                                            s_sb[:, j:j + 1])
            for k in range(NKC):
                for j in range(RJ):
                    pe_t(xt_b[:, k, j * P:(j + 1) * P],
                         xrow_b[:, j, k * P:(k + 1) * P])

        # vr = diag(clip d) @ w^T[:, own fblk]  (f32r rounded on the scale)
        vr_raw = rowsp.tile([P, DKC, FBR], F32)
        nc.sync.dma_start(vr_raw[:],
                          w_colT.ap().rearrange("(k p) n -> p k n", p=P))
        vr = rowsp.tile([P, DKC, FBR], F32R)
        for k in range(DKC):
            nc.vector.tensor_scalar_mul(vr[:, k, :], vr_raw[:, k, :],
                                        d_sb[:, k:k + 1])

        # =========================================================
        # w_mat pass (fp32r): xr_b = 0.1*(w diag(d) w^T - I) col block
        # =========================================================
        xr_b = rowsp.tile([P, DKC, FBR], BF16)
        xr_row_b = rowsp.tile([P, D], BF16)
        for m in range(DKC):
            sl_raw = slabp.tile([P, DKC, P], F32, tag="wslab_raw", bufs=2,
                                name="wslab_raw")
            nc.sync.dma_start(
                sl_raw[:],
                wT_full[:, m * P:(m + 1) * P].rearrange("(k p) n -> p k n", p=P))
            sl = slabp.tile([P, DKC, P], F32R, tag="wslab", bufs=2,
                            name="wslab")
            nc.vector.tensor_copy(sl[:], sl_raw[:])
            ps = psum.tile([P, FBR], F32, tag="mmr", bufs=2, name="ps_mmr")
            for k in range(DKC):
                nc.tensor.matmul(ps[:], sl[:, k, :], vr[:, k, :],
                                 start=(k == 0), stop=(k == DKC - 1))
            nc.vector.tensor_scalar_mul(xr_b[:, m, :], ps[:], 0.1)
            nc.vector.scalar_tensor_tensor(xr_b[:, m, :], idn01[:],
                                           mR[:, m:m + 1], xr_b[:, m, :],
                                           AL.mult, AL.add)
            pe_t(xr_row_b[:, m * P:(m + 1) * P], xr_b[:, m, :])
        ccin_xr = dram.tile([DKC * P, P], BF16, name="ccin_xr")
        nc.sync.dma_start(
            ccin_xr[:].rearrange("(t p) n -> p t n", p=P), xr_row_b[:])
        g_xr = dram.tile([N_CORES * DKC * P, P], BF16, addr_space="Shared",
                         name="g_xr")
        nc.gpsimd.collective_compute(
            "AllGather", AL.bypass, replica_groups=LGROUP,
            ins=[ccin_xr.opt()], outs=[g_xr.opt()])

        # =========================================================
        # X^2 pass: lhsT = scaled adjmi slabs (local, replicated input)
        #   Y^T col chunks -> bf16 + transposed rows -> 2 half-gathers
        # =========================================================
        x2t_b = ser.tile([P, NKC, FB], BF16)
        x2row_b = rowsp.tile([P, RJ, N], BF16)
        g_x2 = []

        def x2_half(h):
            for m in range(h * (NKC // 2), (h + 1) * (NKC // 2)):
                sl = slabp.tile([P, NKC, P], BF16, tag="slab", bufs=3,
                                name="slab")
                nc.sync.dma_start(
                    sl[:],
                    adjmi_bf[:, m * P:(m + 1) * P].rearrange(
                        "(k p) n -> p k n", p=P))
                for k in range(NKC):
                    nc.vector.tensor_scalar_mul(sl[:, k, :], sl[:, k, :],
                                                s_full[:, k:k + 1])
                ps = psum.tile([P, FB], F32, tag="mm0", bufs=2, name="ps_mm0")
                for k in range(NKC):
                    nc.tensor.matmul(ps[:], sl[:, k, :], xt_b[:, k, :],
                                     start=(k == 0), stop=(k == NKC - 1))
                nc.vector.tensor_copy(x2t_b[:, m, :], ps[:])
                for j in range(RJ):
                    pe_t(x2row_b[:, j, m * P:(m + 1) * P],
                         x2t_b[:, m, j * P:(j + 1) * P])
            ccin = dram.tile([RJ * (NKC // 2) * P, P], BF16,
                             name=f"ccin_x2{h}")
            h0 = h * (NKC // 2) * P
            nt = NKC // 2
            for j in range(RJ):
                nc.sync.dma_start(
                    ccin[j * nt * P:(j + 1) * nt * P, :].rearrange(
                        "(t p) n -> p t n", p=P),
                    x2row_b[:, j, h0:h0 + nt * P].rearrange(
                        "p (t n) -> p t n", n=P))
            g = dram.tile([N_CORES * RJ * (NKC // 2) * P, P], BF16,
                          addr_space="Shared", name=f"g_x2{h}")
            nc.gpsimd.collective_compute(
                "AllGather", AL.bypass, replica_groups=LGROUP,
                ins=[ccin.opt()], outs=[g.opt()])
            g_x2.append(g)

        x2_half(0)

        # ---- Xr^2 pass (bf16, lhsT = gathered Xr) ----
        xr2_b = rowsp.tile([P, DKC, FBR], BF16)
        xr2_row_b = rowsp.tile([P, D], BF16)
        for m in range(DKC):
            sl = slabp.tile([P, DKC, P], BF16, tag="rslab", bufs=2,
                            name="rslab")
            nc.sync.dma_start(
                sl[:],
                g_xr[:].rearrange("(c t p) n -> p c t n", c=N_CORES,
                                  p=P)[:, :, m, :])
            ps = psum.tile([P, FBR], F32, tag="mmr", bufs=2, name="ps_mmr")
            for k in range(DKC):
                nc.tensor.matmul(ps[:], sl[:, k, :], xr_b[:, k, :],
                                 start=(k == 0), stop=(k == DKC - 1))
            nc.vector.tensor_copy(xr2_b[:, m, :], ps[:])
            pe_t(xr2_row_b[:, m * P:(m + 1) * P], xr2_b[:, m, :])
        ccin_xr2 = dram.tile([DKC * P, P], BF16, name="ccin_xr2")
        nc.sync.dma_start(
            ccin_xr2[:].rearrange("(t p) n -> p t n", p=P), xr2_row_b[:])
        g_xr2 = dram.tile([N_CORES * DKC * P, P], BF16, addr_space="Shared",
                          name="g_xr2")
        nc.gpsimd.collective_compute(
            "AllGather", AL.bypass, replica_groups=LGROUP,
            ins=[ccin_xr2.opt()], outs=[g_xr2.opt()])

        x2_half(1)

        # =========================================================
        # E/P combos (bf16):
        #   te = I/2 + X/6 + Y/24 ; tp = 0.1*(I/6 + X/24 + Y/120)
        # =========================================================
        te_b = ser.tile([P, NKC, FB], BF16)
        tp_b = ser.tile([P, NKC, FB], BF16)
        for m in range(NKC):
            tmp_c = scrp.tile([P, FB], BF16, tag="combo", bufs=3, name="combo")
            nc.vector.tensor_scalar_mul(tmp_c[:], xt_b[:, m, :], 1.0 / 6.0)
            nc.vector.scalar_tensor_tensor(te_b[:, m, :], x2t_b[:, m, :],
                                           1.0 / 24.0, tmp_c[:],
                                           AL.mult, AL.add)
            nc.vector.tensor_scalar_mul(tmp_c[:], xt_b[:, m, :], 0.1 / 24.0)
            nc.vector.scalar_tensor_tensor(tp_b[:, m, :], x2t_b[:, m, :],
                                           0.1 / 120.0, tmp_c[:],
                                           AL.mult, AL.add)
            for h in range(2):
                hs = slice(h * P, (h + 1) * P)
                nc.vector.scalar_tensor_tensor(
                    te_b[:, m, hs], id_te[:], mL[:, 2 * m + h:2 * m + h + 1],
                    te_b[:, m, hs], AL.mult, AL.add)
                nc.vector.scalar_tensor_tensor(
                    tp_b[:, m, hs], id_tp[:], mL[:, 2 * m + h:2 * m + h + 1],
                    tp_b[:, m, hs], AL.mult, AL.add)

        # =========================================================
        # E/P pass: elt = Y*te + X ; m2t = Y*tp + 0.05X + 0.1I
        # =========================================================
        elt_b = main.tile([P, NKC, FB], BF16)
        m2t_b = m2p.tile([P, NKC, FB], BF16)

        def ep_chunk(m):
            h, mm = (0, m) if m < NKC // 2 else (1, m - NKC // 2)
            sl = slabp.tile([P, N_CORES, RJ, P], BF16, tag="slab", bufs=3,
                            name="slab")
            nc.sync.dma_start(
                sl[:],
                g_x2[h][:].rearrange("(c j t p) n -> p c j t n",
                                     c=N_CORES, j=RJ, p=P)[:, :, :, mm, :])
            ps0 = psum.tile([P, FB], F32, tag="mm0", bufs=2, name="ps_mm0")
            ps1 = psum.tile([P, FB], F32, tag="mm1", bufs=2, name="ps_mm1")
            for k in range(NKC):
                lt = sl[:, k // RJ, k % RJ, :]
                nc.tensor.matmul(ps0[:], lt, te_b[:, k, :],
                                 start=(k == 0), stop=(k == NKC - 1))
                nc.tensor.matmul(ps1[:], lt, tp_b[:, k, :],
                                 start=(k == 0), stop=(k == NKC - 1))
            nc.vector.scalar_tensor_tensor(elt_b[:, m, :], xt_b[:, m, :],
                                           1.0, ps0[:], AL.mult, AL.add)
            nc.vector.scalar_tensor_tensor(m2t_b[:, m, :], xt_b[:, m, :],
                                           0.05, ps1[:], AL.mult, AL.add)
            for h2 in range(2):
                hs = slice(h2 * P, (h2 + 1) * P)
                nc.vector.scalar_tensor_tensor(
                    m2t_b[:, m, hs], id_m2[:],
                    mL[:, 2 * m + h2:2 * m + h2 + 1],
                    m2t_b[:, m, hs], AL.mult, AL.add)

        for m in range(NKC // 2):
            ep_chunk(m)

        # ---- E_R pass (between E/P halves): er = Yr*tr + Xr ----
        tr_b = rowsp.tile([P, DKC, FBR], BF16, name="tr_b")
        for m in range(DKC):
            nc.vector.tensor_scalar_mul(tr_b[:, m, :], xr_b[:, m, :],
                                        1.0 / 6.0)
            nc.vector.scalar_tensor_tensor(tr_b[:, m, :], xr2_b[:, m, :],
                                           1.0 / 24.0, tr_b[:, m, :],
                                           AL.mult, AL.add)
            nc.vector.scalar_tensor_tensor(tr_b[:, m, :], id_te[:],
                                           mR[:, m:m + 1], tr_b[:, m, :],
                                           AL.mult, AL.add)
        er_row_b = rowsp.tile([P, D], BF16)
        for m in range(DKC):
            sl = slabp.tile([P, DKC, P], BF16, tag="rslab", bufs=2,
                            name="rslab")
            nc.sync.dma_start(
                sl[:],
                g_xr2[:].rearrange("(c t p) n -> p c t n", c=N_CORES,
                                   p=P)[:, :, m, :])
            ps = psum.tile([P, FBR], F32, tag="mmr", bufs=2, name="ps_mmr")
            for k in range(DKC):
                nc.tensor.matmul(ps[:], sl[:, k, :], tr_b[:, k, :],
                                 start=(k == 0), stop=(k == DKC - 1))
            erc = scrp.tile([P, FBR], BF16, tag="erc", bufs=2, name="erc")
            nc.vector.scalar_tensor_tensor(erc[:], xr_b[:, m, :], 1.0,
                                           ps[:], AL.mult, AL.add)
            pe_t(er_row_b[:, m * P:(m + 1) * P], erc[:])
        ccin_er = dram.tile([DKC * P, P], BF16, name="ccin_er")
        nc.sync.dma_start(
            ccin_er[:].rearrange("(t p) n -> p t n", p=P), er_row_b[:])
        g_er = dram.tile([N_CORES * DKC * P, P], BF16, addr_space="Shared",
                         name="g_er")
        nc.gpsimd.collective_compute(
            "AllGather", AL.bypass, replica_groups=LGROUP,
            ins=[ccin_er.opt()], outs=[g_er.opt()])

        for m in range(NKC // 2, NKC):
            ep_chunk(m)

        rows_st.close()

        # ---- forcing: ft = (m2 @ x0)^T col block (fp32) ----
        ft = main.tile([P, DKC, FB], F32)
        for m in range(DKC):
            sl = slabp.tile([P, NKC, P], BF16, tag="slab", bufs=3,
                            name="slab")
            nc.sync.dma_start(
                sl[:],
                x0_bf[:, m * P:(m + 1) * P].rearrange("(k p) n -> p k n", p=P))
            ps = psum.tile([P, FB], F32, tag="mm0", bufs=2, name="ps_mm0")
            for k in range(NKC):
                nc.tensor.matmul(ps[:], sl[:, k, :], m2t_b[:, k, :],
                                 start=(k == 0), stop=(k == NKC - 1))
            nc.vector.tensor_copy(ft[:, m, :], ps[:])
        m2_st.close()
        ser_st.close()

        # ---- E_R full into SBUF ----
        er_sb = main.tile([P, DKC, DKC, P], BF16)
        nc.sync.dma_start(
            er_sb[:],
            g_er[:].rearrange("(c t p) n -> p c t n", c=N_CORES, p=P))

        # ---- IC^T col block init (fp32) ----
        icp = [main.tile([P, DKC, FB], F32, name=f"ict{i}") for i in range(2)]
        nc.sync.dma_start(
            icp[0][:], x_colT.ap().rearrange("(q p) n -> p q n", p=P))

        # =========================================================
        # recurrence: V = IC + E_L@IC ; IC' = V + V@E_R + F
        # =========================================================
        ic_g = None   # list of 2 half-gathers when t > 0
        for t in range(NSTEPS):
            ict = icp[t % 2]
            icnt = icp[(t + 1) % 2]
            v = main.tile([P, DKC, FB], F32, tag="v", bufs=1, name="v")
            v_b = main.tile([P, DKC, FB], BF16, tag="v_b", bufs=1, name="v_b")
            for m in range(DKC):
                if t == 0:
                    sl = slabp.tile([P, NKC, P], BF16, tag="icslab",
                                    bufs=3, name="slab")
                    nc.sync.dma_start(
                        sl[:],
                        x_bf[:, m * P:(m + 1) * P].rearrange(
                            "(k p) n -> p k n", p=P))
                    lt = lambda k: sl[:, k, :]
                else:
                    h, mm = (0, m) if m < DKC // 2 else (1, m - DKC // 2)
                    sl = slabp.tile([P, N_CORES, RJ, P], BF16,
                                    tag="slab", bufs=3, name="slab")
                    nc.sync.dma_start(
                        sl[:],
                        ic_g[h][:].rearrange(
                            "(c j t p) n -> p c j t n", c=N_CORES, j=RJ,
                            p=P)[:, :, :, mm, :])
                    lt = lambda k: sl[:, k // RJ, k % RJ, :]
                ps = psum.tile([P, FB], F32, tag="mm0", bufs=2, name="ps_mm0")
                for k in range(NKC):
                    nc.tensor.matmul(ps[:], lt(k), elt_b[:, k, :],
                                     start=(k == 0), stop=(k == NKC - 1))
                nc.vector.scalar_tensor_tensor(v[:, m, :], ict[:, m, :],
                                               1.0, ps[:], AL.mult, AL.add)
                nc.vector.tensor_copy(v_b[:, m, :], v[:, m, :])

            icrow_b = main.tile([P, RJ, D], BF16, tag="icrow", bufs=2,
                                name="icrow")
            for m in range(DKC):
                ps = psum.tile([P, FB], F32, tag="mm1", bufs=2, name="ps_mm1")
                for k in range(DKC):
                    nc.tensor.matmul(ps[:], er_sb[:, k, m, :], v_b[:, k, :],
                                     start=(k == 0), stop=(k == DKC - 1))
                nc.vector.scalar_tensor_tensor(icnt[:, m, :], v[:, m, :],
                                               1.0, ps[:], AL.mult, AL.add)
                nc.vector.scalar_tensor_tensor(icnt[:, m, :], ft[:, m, :],
                                               1.0, icnt[:, m, :],
                                               AL.mult, AL.add)
                if t < NSTEPS - 1:
                    for j in range(RJ):
                        pe_t(icrow_b[:, j, m * P:(m + 1) * P],
                             icnt[:, m, j * P:(j + 1) * P])
                    if m == DKC // 2 - 1 or m == DKC - 1:
                        h = 0 if m < DKC // 2 else 1
                        h0 = h * (DKC // 2) * P
                        ccin = dram.tile([RJ * (DKC // 2) * P, P], BF16,
                                         tag="ccin_ic", name=f"ccin_ic{t}_{h}")
                        nq = DKC // 2
                        for j in range(RJ):
                            nc.sync.dma_start(
                                ccin[j * nq * P:(j + 1) * nq * P, :].rearrange(
                                    "(q p) n -> p q n", p=P),
                                icrow_b[:, j, h0:h0 + nq * P].rearrange(
                                    "p (q n) -> p q n", n=P))
                        g = dram.tile([N_CORES * RJ * (DKC // 2) * P, P],
                                      BF16, addr_space="Shared",
                                      name=f"g_ic{t}_{h}")
                        nc.gpsimd.collective_compute(
                            "AllGather", AL.bypass, replica_groups=LGROUP,
                            ins=[ccin.opt()], outs=[g.opt()])
                        if m == DKC // 2 - 1:
                            ic_g = [g]
                        else:
                            ic_g.append(g)

        # ---- output: z = IC_9 rows (fp32) ----
        zrow = main.tile([P, RJ, D], F32, name="zrow")
        icfin = icp[NSTEPS % 2]
        for m in range(DKC):
            for j in range(RJ):
                pe_t(zrow[:, j, m * P:(m + 1) * P],
                     icfin[:, m, j * P:(j + 1) * P])
        for j in range(RJ):
            nc.sync.dma_start(z_loc[j * P:(j + 1) * P, :], zrow[:, j, :])

    nc.compile()
    return nc


_NC_CACHE = []


def _get_nc():
    if not _NC_CACHE:
        _NC_CACHE.append(build_nc())
    return _NC_CACHE[0]


def make_in_maps(inputs):
    x = np.ascontiguousarray(np.asarray(inputs["x"], dtype=np.float32))
    x0 = np.ascontiguousarray(np.asarray(inputs["x0"], dtype=np.float32))
    adj = np.ascontiguousarray(np.asarray(inputs["adj"], dtype=np.float32))
    alpha = np.ascontiguousarray(np.asarray(inputs["alpha_train"],
                                            dtype=np.float32))
    w = np.ascontiguousarray(np.asarray(inputs["w"], dtype=np.float32))
    d = np.ascontiguousarray(np.asarray(inputs["d"], dtype=np.float32))

    adjmi = adj.copy()
    np.fill_diagonal(adjmi, np.diagonal(adjmi) - 1.0)
    adjmi_b = adjmi.astype(BFNP)
    x_b = x.astype(BFNP)
    x0_b = x0.astype(BFNP)
    wT = np.ascontiguousarray(w.T)

    in_maps = []
    for c in range(N_CORES):
        r0 = c * RB
        f0 = c * FBR
        ml = np.zeros((P, NKC * 2), np.float32)
        ml[:, 2 * (2 * c)] = 1.0          # chunk 2c, half 0
        ml[:, 2 * (2 * c + 1) + 1] = 1.0  # chunk 2c+1, half 1
        mr = np.zeros((P, DKC), np.float32)
        mr[:, c] = 1.0
        in_maps.append({
            "adjmi_rows": np.ascontiguousarray(adjmi_b[r0:r0 + RB, :]),
            "alpha_blk": np.ascontiguousarray(alpha[r0:r0 + RB]),
            "x_colT": np.ascontiguousarray(x[r0:r0 + RB, :].T),
            "w_colT": np.ascontiguousarray(w[f0:f0 + FBR, :].T),
            "masksL": ml,
            "masksR": mr,
            "adjmi_bf": adjmi_b,
            "alpha_full": alpha,
            "x_bf": x_b,
            "x0_bf": x0_b,
            "wT_full": wT,
            "d_full": d,
        })
    return in_maps


def kernel(**inputs) -> np.ndarray:
    nc = _get_nc()
    in_maps = make_in_maps(inputs)
    res = run_bass_kernel_spmd(nc, in_maps, core_ids=list(range(N_CORES)))
    z = np.concatenate([res.results[c]["z_loc"] for c in range(N_CORES)], axis=0)
    return np.ascontiguousarray(z.astype(np.float32))


if __name__ == "__main__":
    rng = np.random.default_rng(0)
    ins = {
        "x": rng.standard_normal((N, D)).astype(np.float32),
        "x0": rng.standard_normal((N, D)).astype(np.float32),
        "adj": (rng.random((N, N)) / N).astype(np.float32),
        "alpha_train": rng.standard_normal((N,)).astype(np.float32),
        "w": (np.eye(D) + 0.02 * rng.standard_normal((D, D))).astype(np.float32),
        "d": rng.random((D,)).astype(np.float32),
    }
    out = kernel(**ins)
    print("kernel output:", out.shape, out.dtype, float(np.linalg.norm(out)))


# revision 13
# speedup vs baseline: 2.3230x; 1.2997x over previous
"""Trainium2 Bass kernel for the ETD1 ODE block (nn_ODEblockW_28922309771809).

Math (mirrors the jax reference; degree-4 Taylor, exact to ~1e-7 here since
||dt*A||_2 ~ 0.05 and ||dt*B||_2 ~ 0.16):
  s    = 0.05 * sigmoid(alpha);  X = diag(s)(adj - I);  Y = X^2
  E_L  = m1_L - I   = X  + Y(I/2 + X/6 + Y/24)
  m2   = dt*phi1(X) = 0.1(I + X/2) + Y*0.1(I/6 + X/24 + Y/120)
  Xr   = 0.1((w*clip(d,0,1)) w^T - I);  Yr = Xr^2
  E_R  = m1_R - I   = Xr + Yr(I/2 + Xr/6 + Yr/24)
  F    = m2 @ x0
  z1   = m1_L x m1_R + F                   # one plain step
then FOUR doubled steps (z_{t+2} = m1_L^2 z_t m1_R^2 + G), using the
identities  m1_L^2 - I = e^{2X} - I  and  m1_L m2 = dt(2 phi1(2X) - phi1(X)),
so every doubled-step operator is just another polynomial in the SAME X, Y:
  E_L2 = 2X + Y(2I + 4X/3 + 2Y/3)
  md   = m1_L m2 = 0.1(I + 1.5X) + Y*0.1(7I/6 + 5X/8 + 31Y/120)
  E_R2 = 2Xr + Yr(2I + 4Xr/3 + 2Yr/3)
  F2   = md @ x0 (= m1_L F);   G = F2 + F2@E_R + F  (= m1_L F m1_R + F)
  step: V = IC + E_L2@IC ; IC <- V + V@E_R2 + G
This cuts the recurrence from 9 gathered steps to 1 + 4 (4 IC AllGathers),
and the four L-series come out of ONE four-rhs matmul pass over gathered Y.

Distribution over 8 cores (transposed-column-local): node dim sharded 256
rows/core, feature dim 128/core; local tensors are transposed column
blocks.  Full matrices needed as matmul lhsT come from replicated DRAM
inputs (adj-I, w^T, x, x0 — no collective) or from bf16 tiled AllGathers
(Y in 2 halves, Xr, E_R, E_R2, and IC each step in 2 halves).  The X^2
pass streams raw (adj-I) slabs and folds the diag(s) row scaling into a
pre-scaled rhs (X^T X^T = M^T diag(s) M^T diag(s), M = adj-I).

Precision: quadratic+ series terms and all matmul inputs bf16; linear
terms and state accumulation fp32.  Measured ~1.4e-3 frob rel err vs the
fp32 reference (tolerance 2e-2).
"""

from contextlib import ExitStack

import numpy as np
import ml_dtypes

import concourse.bass as bass
import concourse.mybir as mybir
import concourse.tile as tile
from concourse import bacc
from concourse.bass_utils import run_bass_kernel_spmd
from concourse.masks import make_identity

F32 = mybir.dt.float32
F32R = mybir.dt.float32r
BF16 = mybir.dt.bfloat16
AL = mybir.AluOpType

N_CORES = 8
P = 128
N = 2048          # nodes
D = 1024          # features
RB = 256          # node row-block per core
FB = 256          # L-side col width (= RB)
FBR = 128         # feature block width per core
NKC = N // P      # 16
DKC = D // P      # 8
RJ = RB // P      # 2
NDOUBLE = 4       # 9 steps = 1 single + 4 doubled

LGROUP = [list(range(N_CORES))]
BFNP = ml_dtypes.bfloat16


def build_nc():
    nc = bacc.Bacc("TRN2", target_bir_lowering=False, debug=False,
                   num_devices=N_CORES)

    # ---- per-core inputs ----
    adjmi_rows = nc.dram_tensor("adjmi_rows", [RB, N], BF16, kind="ExternalInput")
    alpha_blk = nc.dram_tensor("alpha_blk", [RB], F32, kind="ExternalInput")
    x_colT = nc.dram_tensor("x_colT", [D, RB], F32, kind="ExternalInput")
    w_colT = nc.dram_tensor("w_colT", [D, FBR], F32, kind="ExternalInput")
    masksL = nc.dram_tensor("masksL", [P, NKC * 2], F32, kind="ExternalInput")
    masksR = nc.dram_tensor("masksR", [P, DKC], F32, kind="ExternalInput")
    # ---- replicated inputs (same array on every core) ----
    adjmi_bf = nc.dram_tensor("adjmi_bf", [N, N], BF16, kind="ExternalInput")
    alpha_full = nc.dram_tensor("alpha_full", [N], F32, kind="ExternalInput")
    x_bf = nc.dram_tensor("x_bf", [N, D], BF16, kind="ExternalInput")
    x0_bf = nc.dram_tensor("x0_bf", [N, D], BF16, kind="ExternalInput")
    wT_full = nc.dram_tensor("wT_full", [D, D], F32, kind="ExternalInput")
    d_full = nc.dram_tensor("d_full", [D], F32, kind="ExternalInput")
    z_loc = nc.dram_tensor("z_loc", [RB, D], F32, kind="ExternalOutput")

    with tile.TileContext(nc) as tc, ExitStack() as top:
        const = top.enter_context(tc.tile_pool(name="const", bufs=1))
        dram = top.enter_context(tc.tile_pool(name="dram", bufs=1, space="DRAM"))
        psum = top.enter_context(tc.tile_pool(name="psum", bufs=2, space="PSUM"))
        slabp = top.enter_context(tc.tile_pool(name="slabp", bufs=1))
        scrp = top.enter_context(tc.tile_pool(name="scrp", bufs=1))
        main = top.enter_context(tc.tile_pool(name="main", bufs=1))

        ident = const.tile([P, P], F32)
        make_identity(nc, ident)
        ident_b = const.tile([P, P], BF16)
        nc.vector.tensor_copy(ident_b[:], ident[:])
        # scaled identity tiles for masked diagonal adds
        id_te = const.tile([P, P], BF16)       # 0.5 I   (te, tr)
        nc.vector.tensor_scalar_mul(id_te[:], ident[:], 0.5)
        id_tp = const.tile([P, P], BF16)       # (0.1/6) I
        nc.vector.tensor_scalar_mul(id_tp[:], ident[:], 0.1 / 6.0)
        id_m2 = const.tile([P, P], BF16)       # 0.1 I   (m2, md evicts)
        nc.vector.tensor_scalar_mul(id_m2[:], ident[:], 0.1)
        idn01 = const.tile([P, P], BF16)       # -0.1 I  (Xr)
        nc.vector.tensor_scalar_mul(idn01[:], ident[:], -0.1)
        id2 = const.tile([P, P], BF16)         # 2 I     (tq, tqr)
        nc.vector.tensor_scalar_mul(id2[:], ident[:], 2.0)
        id_md = const.tile([P, P], BF16)       # (0.7/6) I  (tmd)
        nc.vector.tensor_scalar_mul(id_md[:], ident[:], 0.7 / 6.0)

        mL = const.tile([P, NKC * 2], F32)
        nc.sync.dma_start(mL[:], masksL[:])
        mR = const.tile([P, DKC], F32)
        nc.sync.dma_start(mR[:], masksR[:])

        def pe_t(dst_slice, src_slice):
            """dst[128,128] = src[128,128].T via PE transpose."""
            if src_slice.dtype == F32R:
                src_slice = src_slice.bitcast(F32)
            fp32_in = src_slice.dtype == F32
            ps = psum.tile([P, P], F32 if fp32_in else BF16, tag="tr",
                           bufs=2, name="ps_tr")
            nc.tensor.transpose(ps[:], src_slice,
                                ident[:] if fp32_in else ident_b[:])
            nc.vector.tensor_copy(dst_slice, ps[:])

        # =========================================================
        # scales
        # =========================================================
        s_sb = const.tile([P, RJ], F32)
        nc.sync.dma_start(s_sb[:], alpha_blk.ap().rearrange("(j p) -> p j", p=P))
        nc.scalar.activation(s_sb[:], s_sb[:],
                             mybir.ActivationFunctionType.Sigmoid)
        nc.vector.tensor_scalar_mul(s_sb[:], s_sb[:], 0.05)

        s_full = const.tile([P, NKC], F32)
        nc.sync.dma_start(s_full[:], alpha_full.ap().rearrange("(k p) -> p k", p=P))
        nc.scalar.activation(s_full[:], s_full[:],
                             mybir.ActivationFunctionType.Sigmoid)
        nc.vector.tensor_scalar_mul(s_full[:], s_full[:], 0.05)

        d_sb = const.tile([P, DKC], F32)
        nc.sync.dma_start(d_sb[:], d_full.ap().rearrange("(q p) -> p q", p=P))
        nc.vector.tensor_scalar(d_sb[:], d_sb[:], 0.0, 1.0, AL.max, AL.min)

        # =========================================================
        # phase pools (stack: ser -> m2p -> rows; close rows, m2p, ser)
        # =========================================================
        ser_st = ExitStack()
        ser = ser_st.enter_context(tc.tile_pool(name="ph_ser", bufs=1))
        m2_st = ExitStack()
        m2p = m2_st.enter_context(tc.tile_pool(name="ph_m2", bufs=1))
        rows_st = ExitStack()
        rowsp = rows_st.enter_context(tc.tile_pool(name="ph_rows", bufs=1))

        # ---- xt_b = X^T[:, own 256 cols]; xts_b = diag(s) X^T ----
        xt_b = ser.tile([P, NKC, FB], BF16)
        xts_b = ser.tile([P, NKC, FB], BF16)
        with tc.tile_pool(name="ph_x", bufs=1) as ph_x:
            xrow_b = ph_x.tile([P, RJ, N], BF16)
            for j in range(RJ):
                raw = scrp.tile([P, N], BF16, tag="adj_in", bufs=1, name="adj_in")
                nc.sync.dma_start(raw[:], adjmi_rows[j * P:(j + 1) * P, :])
                nc.vector.tensor_scalar_mul(xrow_b[:, j, :], raw[:],
                                            s_sb[:, j:j + 1])
            for k in range(NKC):
                for j in range(RJ):
                    pe_t(xt_b[:, k, j * P:(j + 1) * P],
                         xrow_b[:, j, k * P:(k + 1) * P])
        for k in range(NKC):
            nc.vector.tensor_scalar_mul(xts_b[:, k, :], xt_b[:, k, :],
                                        s_full[:, k:k + 1])

        # =========================================================
        # X^2 pass (raw adj-I slabs, pre-scaled rhs).  Per chunk the psum
        # (= Y^T chunk, fp32) feeds the four series combos directly.
        # =========================================================
        te_b = ser.tile([P, NKC, FB], BF16)
        tp_b = ser.tile([P, NKC, FB], BF16)
        tq_b = ser.tile([P, NKC, FB], BF16)
        tmd_b = ser.tile([P, NKC, FB], BF16)
        x2row_b = rowsp.tile([P, RJ, N], BF16)
        g_x2 = []

        COMBOS = [(te_b, 1.0 / 6.0, 1.0 / 24.0, id_te),
                  (tp_b, 0.1 / 24.0, 0.1 / 120.0, id_tp),
                  (tq_b, 4.0 / 3.0, 2.0 / 3.0, id2),
                  (tmd_b, 0.0625, 0.1 * 31.0 / 120.0, id_md)]

        def x2_chunk(m):
            sl = slabp.tile([P, NKC, P], BF16, tag="slab", bufs=3, name="slab")
            nc.sync.dma_start(
                sl[:],
                adjmi_bf[:, m * P:(m + 1) * P].rearrange("(k p) n -> p k n", p=P))
            ps = psum.tile([P, FB], F32, tag="mm0", bufs=2, name="ps_mm0")
            for k in range(NKC):
                nc.tensor.matmul(ps[:], sl[:, k, :], xts_b[:, k, :],
                                 start=(k == 0), stop=(k == NKC - 1))
            x2c = scrp.tile([P, FB], BF16, tag="x2c", bufs=2, name="x2c")
            nc.vector.tensor_copy(x2c[:], ps[:])
            for dst, c1, c2, idt in COMBOS:
                tmp = scrp.tile([P, FB], BF16, tag="combo", bufs=2, name="combo")
                nc.vector.tensor_scalar_mul(tmp[:], xt_b[:, m, :], c1)
                nc.vector.scalar_tensor_tensor(dst[:, m, :], ps[:], c2,
                                               tmp[:], AL.mult, AL.add)
                for h in range(2):
                    hs = slice(h * P, (h + 1) * P)
                    nc.vector.scalar_tensor_tensor(
                        dst[:, m, hs], idt[:], mL[:, 2 * m + h:2 * m + h + 1],
                        dst[:, m, hs], AL.mult, AL.add)
            for j in range(RJ):
                pe_t(x2row_b[:, j, m * P:(m + 1) * P],
                     x2c[:, j * P:(j + 1) * P])

        def x2_gather(h):
            ccin = dram.tile([RJ * (NKC // 2) * P, P], BF16, name=f"ccin_x2{h}")
            h0 = h * (NKC // 2) * P
            nt = NKC // 2
            for j in range(RJ):
                nc.sync.dma_start(
                    ccin[j * nt * P:(j + 1) * nt * P, :].rearrange(
                        "(t p) n -> p t n", p=P),
                    x2row_b[:, j, h0:h0 + nt * P].rearrange(
                        "p (t n) -> p t n", n=P))
            g = dram.tile([N_CORES * RJ * (NKC // 2) * P, P], BF16,
                          addr_space="Shared", name=f"g_x2{h}")
            nc.gpsimd.collective_compute(
                "AllGather", AL.bypass, replica_groups=LGROUP,
                ins=[ccin.opt()], outs=[g.opt()])
            g_x2.append(g)

        for m in range(NKC // 2):
            x2_chunk(m)
        x2_gather(0)

        # ---- w_mat pass (fp32r): xr_b = 0.1(w diag(d) w^T - I) ----
        vr_raw = rowsp.tile([P, DKC, FBR], F32)
        nc.sync.dma_start(vr_raw[:],
                          w_colT.ap().rearrange("(k p) n -> p k n", p=P))
        vr = rowsp.tile([P, DKC, FBR], F32R)
        for k in range(DKC):
            nc.vector.tensor_scalar_mul(vr[:, k, :], vr_raw[:, k, :],
                                        d_sb[:, k:k + 1])
        xr_b = rowsp.tile([P, DKC, FBR], BF16)
        xr_row_b = rowsp.tile([P, D], BF16)
        for m in range(DKC):
            sl_raw = slabp.tile([P, DKC, P], F32, tag="wslab_raw", bufs=2,
                                name="wslab_raw")
            nc.sync.dma_start(
                sl_raw[:],
                wT_full[:, m * P:(m + 1) * P].rearrange("(k p) n -> p k n", p=P))
            sl = slabp.tile([P, DKC, P], F32R, tag="wslab", bufs=2,
                            name="wslab")
            nc.vector.tensor_copy(sl[:], sl_raw[:])
            ps = psum.tile([P, FB], F32, tag="mm1", bufs=2, name="ps_mm1")
            for k in range(DKC):
                nc.tensor.matmul(ps[:, :FBR], sl[:, k, :], vr[:, k, :],
                                 start=(k == 0), stop=(k == DKC - 1))
            nc.vector.tensor_scalar_mul(xr_b[:, m, :], ps[:, :FBR], 0.1)
            nc.vector.scalar_tensor_tensor(xr_b[:, m, :], idn01[:],
                                           mR[:, m:m + 1], xr_b[:, m, :],
                                           AL.mult, AL.add)
            pe_t(xr_row_b[:, m * P:(m + 1) * P], xr_b[:, m, :])
        ccin_xr = dram.tile([DKC * P, P], BF16, name="ccin_xr")
        nc.sync.dma_start(
            ccin_xr[:].rearrange("(t p) n -> p t n", p=P), xr_row_b[:])
        g_xr = dram.tile([N_CORES * DKC * P, P], BF16, addr_space="Shared",
                         name="g_xr")
        nc.gpsimd.collective_compute(
            "AllGather", AL.bypass, replica_groups=LGROUP,
            ins=[ccin_xr.opt()], outs=[g_xr.opt()])

        for m in range(NKC // 2, NKC):
            x2_chunk(m)
        x2_gather(1)

        # =========================================================
        # E/P pass, four series at once:
        #   elt = Y te + X            m2t = Y tp + .05X + .1I
        #   el2t = Y tq + 2X          mdt = Y tmd + .15X + .1I
        # =========================================================
        elt_b = main.tile([P, NKC, FB], BF16)
        el2t_b = main.tile([P, NKC, FB], BF16)
        m2t_b = m2p.tile([P, NKC, FB], BF16)
        mdt_b = m2p.tile([P, NKC, FB], BF16)

        def ep_chunk(m):
            h, mm = (0, m) if m < NKC // 2 else (1, m - NKC // 2)
            sl = slabp.tile([P, N_CORES, RJ, P], BF16, tag="slab", bufs=3,
                            name="slab")
            nc.sync.dma_start(
                sl[:],
                g_x2[h][:].rearrange("(c j t p) n -> p c j t n",
                                     c=N_CORES, j=RJ, p=P)[:, :, :, mm, :])
            ps0 = psum.tile([P, FB], F32, tag="mm0", bufs=2, name="ps_mm0")
            ps1 = psum.tile([P, FB], F32, tag="mm1", bufs=2, name="ps_mm1")
            ps2 = psum.tile([P, FB], F32, tag="mm2", bufs=1, name="ps_mm2")
            ps3 = psum.tile([P, FB], F32, tag="mm3", bufs=1, name="ps_mm3")
            for k in range(NKC):
                lt = sl[:, k // RJ, k % RJ, :]
                nc.tensor.matmul(ps0[:], lt, te_b[:, k, :],
                                 start=(k == 0), stop=(k == NKC - 1))
                nc.tensor.matmul(ps1[:], lt, tp_b[:, k, :],
                                 start=(k == 0), stop=(k == NKC - 1))
                nc.tensor.matmul(ps2[:], lt, tq_b[:, k, :],
                                 start=(k == 0), stop=(k == NKC - 1))
                nc.tensor.matmul(ps3[:], lt, tmd_b[:, k, :],
                                 start=(k == 0), stop=(k == NKC - 1))
            nc.vector.scalar_tensor_tensor(elt_b[:, m, :], xt_b[:, m, :],
                                           1.0, ps0[:], AL.mult, AL.add)
            nc.vector.scalar_tensor_tensor(el2t_b[:, m, :], xt_b[:, m, :],
                                           2.0, ps2[:], AL.mult, AL.add)
            nc.vector.scalar_tensor_tensor(m2t_b[:, m, :], xt_b[:, m, :],
                                           0.05, ps1[:], AL.mult, AL.add)
            nc.vector.scalar_tensor_tensor(mdt_b[:, m, :], xt_b[:, m, :],
                                           0.15, ps3[:], AL.mult, AL.add)
            for h2 in range(2):
                hs = slice(h2 * P, (h2 + 1) * P)
                msk = mL[:, 2 * m + h2:2 * m + h2 + 1]
                nc.vector.scalar_tensor_tensor(m2t_b[:, m, hs], id_m2[:], msk,
                                               m2t_b[:, m, hs], AL.mult, AL.add)
                nc.vector.scalar_tensor_tensor(mdt_b[:, m, hs], id_m2[:], msk,
                                               mdt_b[:, m, hs], AL.mult, AL.add)

        for m in range(NKC // 2):
            ep_chunk(m)

        # ---- Xr^2 pass (between E/P halves; needs g_xr) ----
        xr2_b = rowsp.tile([P, DKC, FBR], BF16)
        for m in range(DKC):
            sl = slabp.tile([P, DKC, P], BF16, tag="rslab", bufs=2,
                            name="rslab")
            nc.sync.dma_start(
                sl[:],
                g_xr[:].rearrange("(c t p) n -> p c t n", c=N_CORES,
                                  p=P)[:, :, m, :])
            ps = psum.tile([P, FB], F32, tag="mm1", bufs=2, name="ps_mm1")
            for k in range(DKC):
                nc.tensor.matmul(ps[:, :FBR], sl[:, k, :], xr_b[:, k, :],
                                 start=(k == 0), stop=(k == DKC - 1))
            nc.vector.tensor_copy(xr2_b[:, m, :], ps[:, :FBR])

        for m in range(NKC // 2, NKC):
            ep_chunk(m)

        # ---- E_R pass: er = Yr tr + Xr ; er2 = Yr tqr + 2Xr ----
        tr_b = rowsp.tile([P, DKC, FBR], BF16, name="tr_b")
        tqr_b = rowsp.tile([P, DKC, FBR], BF16, name="tqr_b")
        for m in range(DKC):
            nc.vector.tensor_scalar_mul(tr_b[:, m, :], xr_b[:, m, :], 1.0 / 6.0)
            nc.vector.scalar_tensor_tensor(tr_b[:, m, :], xr2_b[:, m, :],
                                           1.0 / 24.0, tr_b[:, m, :],
                                           AL.mult, AL.add)
            nc.vector.scalar_tensor_tensor(tr_b[:, m, :], id_te[:],
                                           mR[:, m:m + 1], tr_b[:, m, :],
                                           AL.mult, AL.add)
            nc.vector.tensor_scalar_mul(tqr_b[:, m, :], xr_b[:, m, :], 4.0 / 3.0)
            nc.vector.scalar_tensor_tensor(tqr_b[:, m, :], xr2_b[:, m, :],
                                           2.0 / 3.0, tqr_b[:, m, :],
                                           AL.mult, AL.add)
            nc.vector.scalar_tensor_tensor(tqr_b[:, m, :], id2[:],
                                           mR[:, m:m + 1], tqr_b[:, m, :],
                                           AL.mult, AL.add)
        # u1 = Xr tr ; u2 = Xr tqr   (lhsT = gathered Xr)
        u1_b = rowsp.tile([P, DKC, FBR], BF16, name="u1_b")
        u2_b = rowsp.tile([P, DKC, FBR], BF16, name="u2_b")
        for m in range(DKC):
            sl = slabp.tile([P, DKC, P], BF16, tag="rslab", bufs=2,
                            name="rslab")
            nc.sync.dma_start(
                sl[:],
                g_xr[:].rearrange("(c t p) n -> p c t n", c=N_CORES,
                                  p=P)[:, :, m, :])
            ps2 = psum.tile([P, FB], F32, tag="mm2", bufs=1, name="ps_mm2")
            ps3 = psum.tile([P, FB], F32, tag="mm3", bufs=1, name="ps_mm3")
            for k in range(DKC):
                nc.tensor.matmul(ps2[:, :FBR], sl[:, k, :], tr_b[:, k, :],
                                 start=(k == 0), stop=(k == DKC - 1))
                nc.tensor.matmul(ps3[:, :FBR], sl[:, k, :], tqr_b[:, k, :],
                                 start=(k == 0), stop=(k == DKC - 1))
            nc.vector.tensor_copy(u1_b[:, m, :], ps2[:, :FBR])
            nc.vector.tensor_copy(u2_b[:, m, :], ps3[:, :FBR])
        # er = Xr + Xr u1 (= Xr + Yr tr) ; er2 = 2Xr + Xr u2
        er_row_b = rowsp.tile([P, D], BF16)
        er2_row_b = rowsp.tile([P, D], BF16)
        for m in range(DKC):
            sl = slabp.tile([P, DKC, P], BF16, tag="rslab", bufs=2,
                            name="rslab")
            nc.sync.dma_start(
                sl[:],
                g_xr[:].rearrange("(c t p) n -> p c t n", c=N_CORES,
                                  p=P)[:, :, m, :])
            ps2 = psum.tile([P, FB], F32, tag="mm2", bufs=1, name="ps_mm2")
            ps3 = psum.tile([P, FB], F32, tag="mm3", bufs=1, name="ps_mm3")
            for k in range(DKC):
                nc.tensor.matmul(ps2[:, :FBR], sl[:, k, :], u1_b[:, k, :],
                                 start=(k == 0), stop=(k == DKC - 1))
                nc.tensor.matmul(ps3[:, :FBR], sl[:, k, :], u2_b[:, k, :],
                                 start=(k == 0), stop=(k == DKC - 1))
            erc = scrp.tile([P, FBR], BF16, tag="erc", bufs=2, name="erc")
            nc.vector.scalar_tensor_tensor(erc[:], xr_b[:, m, :], 1.0,
                                           ps2[:, :FBR], AL.mult, AL.add)
            pe_t(er_row_b[:, m * P:(m + 1) * P], erc[:])
            er2c = scrp.tile([P, FBR], BF16, tag="erc", bufs=2, name="er2c")
            nc.vector.scalar_tensor_tensor(er2c[:], xr_b[:, m, :], 2.0,
                                           ps3[:, :FBR], AL.mult, AL.add)
            pe_t(er2_row_b[:, m * P:(m + 1) * P], er2c[:])
        ccin_er = dram.tile([DKC * P, P], BF16, name="ccin_er")
        nc.sync.dma_start(
            ccin_er[:].rearrange("(t p) n -> p t n", p=P), er_row_b[:])
        g_er = dram.tile([N_CORES * DKC * P, P], BF16, addr_space="Shared",
                         name="g_er")
        nc.gpsimd.collective_compute(
            "AllGather", AL.bypass, replica_groups=LGROUP,
            ins=[ccin_er.opt()], outs=[g_er.opt()])
        ccin_er2 = dram.tile([DKC * P, P], BF16, name="ccin_er2")
        nc.sync.dma_start(
            ccin_er2[:].rearrange("(t p) n -> p t n", p=P), er2_row_b[:])
        g_er2 = dram.tile([N_CORES * DKC * P, P], BF16, addr_space="Shared",
                          name="g_er2")
        nc.gpsimd.collective_compute(
            "AllGather", AL.bypass, replica_groups=LGROUP,
            ins=[ccin_er2.opt()], outs=[g_er2.opt()])

        rows_st.close()

        # ---- forcing: ft = (m2 x0)^T ; f2t = (md x0)^T  (fp32) ----
        ft = main.tile([P, DKC, FB], F32)
        f2t = main.tile([P, DKC, FB], F32)
        f2t_b = main.tile([P, DKC, FB], BF16)
        for m in range(DKC):
            sl = slabp.tile([P, NKC, P], BF16, tag="slab", bufs=3, name="slab")
            nc.sync.dma_start(
                sl[:],
                x0_bf[:, m * P:(m + 1) * P].rearrange("(k p) n -> p k n", p=P))
            ps0 = psum.tile([P, FB], F32, tag="mm0", bufs=2, name="ps_mm0")
            ps1 = psum.tile([P, FB], F32, tag="mm1", bufs=2, name="ps_mm1")
            for k in range(NKC):
                nc.tensor.matmul(ps0[:], sl[:, k, :], m2t_b[:, k, :],
                                 start=(k == 0), stop=(k == NKC - 1))
                nc.tensor.matmul(ps1[:], sl[:, k, :], mdt_b[:, k, :],
                                 start=(k == 0), stop=(k == NKC - 1))
            nc.vector.tensor_copy(ft[:, m, :], ps0[:])
            nc.vector.tensor_copy(f2t[:, m, :], ps1[:])
            nc.vector.tensor_copy(f2t_b[:, m, :], ps1[:])
        m2_st.close()
        ser_st.close()

        # ---- recurrence-phase pool ----
        rec = top.enter_context(tc.tile_pool(name="rec", bufs=1))
        er_sb = rec.tile([P, DKC, DKC, P], BF16)
        nc.sync.dma_start(
            er_sb[:],
            g_er[:].rearrange("(c t p) n -> p c t n", c=N_CORES, p=P))
        er2_sb = rec.tile([P, DKC, DKC, P], BF16)
        nc.sync.dma_start(
            er2_sb[:],
            g_er2[:].rearrange("(c t p) n -> p c t n", c=N_CORES, p=P))

        icp = [rec.tile([P, DKC, FB], F32, name=f"ict{i}") for i in range(2)]
        nc.sync.dma_start(
            icp[0][:], x_colT.ap().rearrange("(q p) n -> p q n", p=P))

        gt = rec.tile([P, DKC, FB], F32)

        # =========================================================
        # recurrence: step 0 single (elt/er/ft), steps 1..4 doubled
        # (el2t/er2/gt).  G computed between z1's V- and R-passes.
        # =========================================================
        ic_g = None
        for t in range(1 + NDOUBLE):
            ict = icp[t % 2]
            icnt = icp[(t + 1) % 2]
            el_rhs = elt_b if t == 0 else el2t_b
            er_lhs = er_sb if t == 0 else er2_sb
            f_add = ft if t == 0 else gt
            v = rec.tile([P, DKC, FB], F32, tag="v", bufs=1, name="v")
            v_b = rec.tile([P, DKC, FB], BF16, tag="v_b", bufs=1, name="v_b")
            for m in range(DKC):
                if t == 0:
                    sl = slabp.tile([P, NKC, P], BF16, tag="slab", bufs=3,
                                    name="slab")
                    nc.sync.dma_start(
                        sl[:],
                        x_bf[:, m * P:(m + 1) * P].rearrange(
                            "(k p) n -> p k n", p=P))
                    lt = lambda k: sl[:, k, :]
                else:
                    h, mm = (0, m) if m < DKC // 2 else (1, m - DKC // 2)
                    sl = slabp.tile([P, N_CORES, RJ, P], BF16,
                                    tag="slab", bufs=3, name="slab")
                    nc.sync.dma_start(
                        sl[:],
                        ic_g[h][:].rearrange(
                            "(c j t p) n -> p c j t n", c=N_CORES, j=RJ,
                            p=P)[:, :, :, mm, :])
                    lt = lambda k: sl[:, k // RJ, k % RJ, :]
                ps = psum.tile([P, FB], F32, tag="mm0", bufs=2, name="ps_mm0")
                for k in range(NKC):
                    nc.tensor.matmul(ps[:], lt(k), el_rhs[:, k, :],
                                     start=(k == 0), stop=(k == NKC - 1))
                nc.vector.scalar_tensor_tensor(v[:, m, :], ict[:, m, :],
                                               1.0, ps[:], AL.mult, AL.add)
                nc.vector.tensor_copy(v_b[:, m, :], v[:, m, :])

            if t == 0:
                # G = F2 + F2@E_R + F  (R-type pass on f2t)
                for m in range(DKC):
                    ps = psum.tile([P, FB], F32, tag="mm1", bufs=2,
                                   name="ps_mm1")
                    for k in range(DKC):
                        nc.tensor.matmul(ps[:], er_sb[:, k, m, :],
                                         f2t_b[:, k, :],
                                         start=(k == 0), stop=(k == DKC - 1))
                    nc.vector.scalar_tensor_tensor(gt[:, m, :], f2t[:, m, :],
                                                   1.0, ps[:], AL.mult, AL.add)
                    nc.vector.scalar_tensor_tensor(gt[:, m, :], ft[:, m, :],
                                                   1.0, gt[:, m, :],
                                                   AL.mult, AL.add)

            icrow_b = rec.tile([P, RJ, D], BF16, tag="icrow", bufs=2,
                               name="icrow")
            for m in range(DKC):
                ps = psum.tile([P, FB], F32, tag="mm1", bufs=2, name="ps_mm1")
                for k in range(DKC):
                    nc.tensor.matmul(ps[:], er_lhs[:, k, m, :], v_b[:, k, :],
                                     start=(k == 0), stop=(k == DKC - 1))
                nc.vector.scalar_tensor_tensor(icnt[:, m, :], v[:, m, :],
                                               1.0, ps[:], AL.mult, AL.add)
                nc.vector.scalar_tensor_tensor(icnt[:, m, :], f_add[:, m, :],
                                               1.0, icnt[:, m, :],
                                               AL.mult, AL.add)
                if t < NDOUBLE:
                    for j in range(RJ):
                        pe_t(icrow_b[:, j, m * P:(m + 1) * P],
                             icnt[:, m, j * P:(j + 1) * P])
                    if m == DKC // 2 - 1 or m == DKC - 1:
                        h = 0 if m < DKC // 2 else 1
                        h0 = h * (DKC // 2) * P
                        nq = DKC // 2
                        ccin = dram.tile([RJ * nq * P, P], BF16,
                                         tag="ccin_ic", name=f"ccin_ic{t}_{h}")
                        for j in range(RJ):
                            nc.sync.dma_start(
                                ccin[j * nq * P:(j + 1) * nq * P, :].rearrange(
                                    "(q p) n -> p q n", p=P),
                                icrow_b[:, j, h0:h0 + nq * P].rearrange(
                                    "p (q n) -> p q n", n=P))
                        g = dram.tile([N_CORES * RJ * nq * P, P],
                                      BF16, addr_space="Shared",
                                      name=f"g_ic{t}_{h}")
                        nc.gpsimd.collective_compute(
                            "AllGather", AL.bypass, replica_groups=LGROUP,
                            ins=[ccin.opt()], outs=[g.opt()])
                        if m == DKC // 2 - 1:
                            ic_g = [g]
                        else:
                            ic_g.append(g)

        # ---- output ----
        zrow = rec.tile([P, RJ, D], F32, name="zrow")
        icfin = icp[(1 + NDOUBLE) % 2]
        for m in range(DKC):
            for j in range(RJ):
                pe_t(zrow[:, j, m * P:(m + 1) * P],
                     icfin[:, m, j * P:(j + 1) * P])
        for j in range(RJ):
            nc.sync.dma_start(z_loc[j * P:(j + 1) * P, :], zrow[:, j, :])

    nc.compile()
    return nc


_NC_CACHE = []


def _get_nc():
    if not _NC_CACHE:
        _NC_CACHE.append(build_nc())
    return _NC_CACHE[0]


def make_in_maps(inputs):
    x = np.ascontiguousarray(np.asarray(inputs["x"], dtype=np.float32))
    x0 = np.ascontiguousarray(np.asarray(inputs["x0"], dtype=np.float32))
    adj = np.ascontiguousarray(np.asarray(inputs["adj"], dtype=np.float32))
    alpha = np.ascontiguousarray(np.asarray(inputs["alpha_train"],
                                            dtype=np.float32))
    w = np.ascontiguousarray(np.asarray(inputs["w"], dtype=np.float32))
    d = np.ascontiguousarray(np.asarray(inputs["d"], dtype=np.float32))

    adjmi = adj.copy()
    np.fill_diagonal(adjmi, np.diagonal(adjmi) - 1.0)
    adjmi_b = adjmi.astype(BFNP)
    x_b = x.astype(BFNP)
    x0_b = x0.astype(BFNP)
    wT = np.ascontiguousarray(w.T)

    in_maps = []
    for c in range(N_CORES):
        r0 = c * RB
        f0 = c * FBR
        ml = np.zeros((P, NKC * 2), np.float32)
        ml[:, 2 * (2 * c)] = 1.0          # chunk 2c, half 0
        ml[:, 2 * (2 * c + 1) + 1] = 1.0  # chunk 2c+1, half 1
        mr = np.zeros((P, DKC), np.float32)
        mr[:, c] = 1.0
        in_maps.append({
            "adjmi_rows": np.ascontiguousarray(adjmi_b[r0:r0 + RB, :]),
            "alpha_blk": np.ascontiguousarray(alpha[r0:r0 + RB]),
            "x_colT": np.ascontiguousarray(x[r0:r0 + RB, :].T),
            "w_colT": np.ascontiguousarray(w[f0:f0 + FBR, :].T),
            "masksL": ml,
            "masksR": mr,
            "adjmi_bf": adjmi_b,
            "alpha_full": alpha,
            "x_bf": x_b,
            "x0_bf": x0_b,
            "wT_full": wT,
            "d_full": d,
        })
    return in_maps


def kernel(**inputs) -> np.ndarray:
    nc = _get_nc()
    in_maps = make_in_maps(inputs)
    res = run_bass_kernel_spmd(nc, in_maps, core_ids=list(range(N_CORES)))
    z = np.concatenate([res.results[c]["z_loc"] for c in range(N_CORES)], axis=0)
    return np.ascontiguousarray(z.astype(np.float32))


if __name__ == "__main__":
    rng = np.random.default_rng(0)
    ins = {
        "x": rng.standard_normal((N, D)).astype(np.float32),
        "x0": rng.standard_normal((N, D)).astype(np.float32),
        "adj": (rng.random((N, N)) / N).astype(np.float32),
        "alpha_train": rng.standard_normal((N,)).astype(np.float32),
        "w": (np.eye(D) + 0.02 * rng.standard_normal((D, D))).astype(np.float32),
        "d": rng.random((D,)).astype(np.float32),
    }
    out = kernel(**ins)
    print("kernel output:", out.shape, out.dtype, float(np.linalg.norm(out)))


# revision 18
# speedup vs baseline: 2.7121x; 1.1675x over previous
"""Trainium2 Bass kernel for the ETD1 ODE block (nn_ODEblockW_28922309771809).

Math (mirrors the jax reference; degree-4 Taylor, exact to ~1e-6 here since
||dt*A||_2 ~ 0.05 and ||dt*B||_2 ~ 0.16):
  s  = 0.05 * sigmoid(alpha);  X = diag(s)(adj - I);  Y = X^2
  Xr = 0.1((w*clip(d,0,1)) w^T - I);  Yr = Xr^2
The 9 recurrence steps  IC <- m1_L IC m1_R + F  (m1_L = e^X, m1_R = e^{Xr},
F = dt*phi1(X) x0) are regrouped as THREE triple steps
  IC <- m1_L^3 IC m1_R^3 + G3,   G3 = F + m1_L F m1_R + m1_L^2 F m1_R^2,
which needs only TWO inter-step IC AllGathers.  Every operator is a
polynomial in the same X, Y (resp. Xr, Yr), via e^{kX}-I and the phi1
identities  m1_L m2 = dt(2 phi1(2X) - phi1(X)),
            m1_L^2 m2 = dt(3 phi1(3X) - 2 phi1(2X)):
  E_L3 = e^{3X}-I = 3X + 4.5Y + Y(4.5X + 3.375Y)
  m2   = 0.1 I + .05X + (0.1/6)Y  + Y*0.1(X/24   + Y/120)
  md   = 0.1 I + .15X + (0.7/6)Y  + Y*0.1(5X/8   + 31Y/120)    # m1_L m2
  md3  = 0.1 I + .25X + (1.9/6)Y  + Y*0.1(65X/24 + 211Y/120)   # m1_L^2 m2
  E_R  = Xr  + Xr(Xr/2? no: Xr u1), u1 = Xr(X r/6+Yr/24) + Xr/2   (chained,
  E_R2 = 2Xr + Xr u2,  u2 = Xr(4Xr/3+2Yr/3)  + 2Xr               no Yr
  E_R3 = 3Xr + Xr u3,  u3 = Xr(4.5Xr+3.375Yr) + 4.5Xr            gather)
  F = m2 x0;  F2 = md x0;  F3 = md3 x0;  G3 = F + F2 + F2@E_R + F3 + F3@E_R2
  step: V = IC + E_L3@IC ; IC <- V + V@E_R3 + G3

Distribution over 8 cores (transposed-column-local): node dim sharded 256
rows/core, feature dim 128/core.  Full matrices needed as matmul lhsT come
from replicated DRAM inputs (adj-I, w^T, x, x0 - no collective) or from
bf16 tiled AllGathers (Y in 2 halves, Xr, E_R, E_R2, E_R3, and IC per step
in 2 halves).  The X^2 pass streams raw (adj-I) slabs and folds the
diag(s) row scaling into a pre-scaled rhs.  The four L-series come out of
ONE four-rhs matmul pass over gathered Y; identity terms of the inner
combos are folded into the evicts as c*Y adds (no masked-eye ops there).

Precision: quadratic+ series terms and all matmul inputs bf16; linear
terms and state accumulation fp32.  Measured ~2.9e-3 frob rel err vs the
fp32 reference (tolerance 2e-2).
"""

from contextlib import ExitStack

import numpy as np
import ml_dtypes

import concourse.bass as bass
import concourse.mybir as mybir
import concourse.tile as tile
from concourse import bacc
from concourse.bass_utils import run_bass_kernel_spmd
from concourse.masks import make_identity

F32 = mybir.dt.float32
F32R = mybir.dt.float32r
BF16 = mybir.dt.bfloat16
AL = mybir.AluOpType

N_CORES = 8
P = 128
N = 2048          # nodes
D = 1024          # features
RB = 256          # node row-block per core
FB = 256          # L-side col width (= RB)
FBR = 128         # feature block width per core
NKC = N // P      # 16
DKC = D // P      # 8
RJ = RB // P      # 2
NTRIPLE = 3       # 9 steps = 3 tripled

LGROUP = [list(range(N_CORES))]
BFNP = ml_dtypes.bfloat16


def build_nc():
    nc = bacc.Bacc("TRN2", target_bir_lowering=False, debug=False,
                   num_devices=N_CORES)

    # ---- per-core inputs ----
    adjmi_rows = nc.dram_tensor("adjmi_rows", [RB, N], BF16, kind="ExternalInput")
    alpha_blk = nc.dram_tensor("alpha_blk", [RB], F32, kind="ExternalInput")
    x_colT = nc.dram_tensor("x_colT", [D, RB], F32, kind="ExternalInput")
    w_colT = nc.dram_tensor("w_colT", [D, FBR], F32, kind="ExternalInput")
    masksL = nc.dram_tensor("masksL", [P, NKC * 2], F32, kind="ExternalInput")
    masksR = nc.dram_tensor("masksR", [P, DKC], F32, kind="ExternalInput")
    # ---- replicated inputs (same array on every core) ----
    adjmi_bf = nc.dram_tensor("adjmi_bf", [N, N], BF16, kind="ExternalInput")
    alpha_full = nc.dram_tensor("alpha_full", [N], F32, kind="ExternalInput")
    x_bf = nc.dram_tensor("x_bf", [N, D], BF16, kind="ExternalInput")
    x0_bf = nc.dram_tensor("x0_bf", [N, D], BF16, kind="ExternalInput")
    wT_full = nc.dram_tensor("wT_full", [D, D], BF16, kind="ExternalInput")
    d_full = nc.dram_tensor("d_full", [D], F32, kind="ExternalInput")
    z_loc = nc.dram_tensor("z_loc", [RB, D], F32, kind="ExternalOutput")

    with tile.TileContext(nc) as tc, ExitStack() as top:
        const = top.enter_context(tc.tile_pool(name="const", bufs=1))
        dram = top.enter_context(tc.tile_pool(name="dram", bufs=1, space="DRAM"))
        psum = top.enter_context(tc.tile_pool(name="psum", bufs=2, space="PSUM"))
        slabp = top.enter_context(tc.tile_pool(name="slabp", bufs=1))
        scrp = top.enter_context(tc.tile_pool(name="scrp", bufs=1))
        main = top.enter_context(tc.tile_pool(name="main", bufs=1))

        ident = const.tile([P, P], F32)
        make_identity(nc, ident)
        ident_b = const.tile([P, P], BF16)
        nc.vector.tensor_copy(ident_b[:], ident[:])
        # scaled identity tiles for masked diagonal adds
        id_m2 = const.tile([P, P], BF16)       # 0.1 I   (m2/md/md3 evicts)
        nc.vector.tensor_scalar_mul(id_m2[:], ident[:], 0.1)
        idn01 = const.tile([P, P], BF16)       # -0.1 I  (Xr)
        nc.vector.tensor_scalar_mul(idn01[:], ident[:], -0.1)

        mL = const.tile([P, NKC * 2], F32)
        nc.sync.dma_start(mL[:], masksL[:])
        mR = const.tile([P, DKC], F32)
        nc.sync.dma_start(mR[:], masksR[:])

        def pe_t(dst_slice, src_slice):
            """dst[128,128] = src[128,128].T via PE transpose."""
            if src_slice.dtype == F32R:
                src_slice = src_slice.bitcast(F32)
            fp32_in = src_slice.dtype == F32
            ps = psum.tile([P, P], F32 if fp32_in else BF16, tag="tr",
                           bufs=2, name="ps_tr")
            nc.tensor.transpose(ps[:], src_slice,
                                ident[:] if fp32_in else ident_b[:])
            nc.vector.tensor_copy(dst_slice, ps[:])

        # =========================================================
        # scales
        # =========================================================
        s_sb = const.tile([P, RJ], F32)
        nc.sync.dma_start(s_sb[:], alpha_blk.ap().rearrange("(j p) -> p j", p=P))
        nc.scalar.activation(s_sb[:], s_sb[:],
                             mybir.ActivationFunctionType.Sigmoid)
        nc.vector.tensor_scalar_mul(s_sb[:], s_sb[:], 0.05)

        s_full = const.tile([P, NKC], F32)
        nc.sync.dma_start(s_full[:], alpha_full.ap().rearrange("(k p) -> p k", p=P))
        nc.scalar.activation(s_full[:], s_full[:],
                             mybir.ActivationFunctionType.Sigmoid)
        nc.vector.tensor_scalar_mul(s_full[:], s_full[:], 0.05)

        d_sb = const.tile([P, DKC], F32)
        nc.sync.dma_start(d_sb[:], d_full.ap().rearrange("(q p) -> p q", p=P))
        nc.vector.tensor_scalar(d_sb[:], d_sb[:], 0.0, 1.0, AL.max, AL.min)

        # =========================================================
        # phase pools (stack: ser -> m2p -> rows; close rows, m2p, ser)
        # =========================================================
        ser_st = ExitStack()
        ser = ser_st.enter_context(tc.tile_pool(name="ph_ser", bufs=1))
        m2_st = ExitStack()
        m2p = m2_st.enter_context(tc.tile_pool(name="ph_m2", bufs=1))
        rows_st = ExitStack()
        rowsp = rows_st.enter_context(tc.tile_pool(name="ph_rows", bufs=1))

        # ---- xt_b = X^T[:, own 256 cols]; xts_b = diag(s) X^T ----
        xt_b = ser.tile([P, NKC, FB], BF16)
        xts_b = ser.tile([P, NKC, FB], BF16)
        with tc.tile_pool(name="ph_x", bufs=1) as ph_x:
            xrow_b = ph_x.tile([P, RJ, N], BF16)
            for j in range(RJ):
                raw = scrp.tile([P, N], BF16, tag="adj_in", bufs=1, name="adj_in")
                nc.sync.dma_start(raw[:], adjmi_rows[j * P:(j + 1) * P, :])
                nc.vector.tensor_scalar_mul(xrow_b[:, j, :], raw[:],
                                            s_sb[:, j:j + 1])
            for k in range(NKC):
                for j in range(RJ):
                    pe_t(xt_b[:, k, j * P:(j + 1) * P],
                         xrow_b[:, j, k * P:(k + 1) * P])
        for k in range(NKC):
            nc.vector.tensor_scalar_mul(xts_b[:, k, :], xt_b[:, k, :],
                                        s_full[:, k:k + 1])

        # =========================================================
        # X^2 pass (raw adj-I slabs, pre-scaled rhs).  Per chunk the psum
        # (= Y^T chunk, fp32) feeds the four series combos directly.
        # =========================================================
        x2t_b = ser.tile([P, NKC, FB], BF16)
        tp_b = ser.tile([P, NKC, FB], BF16)
        tmd_b = ser.tile([P, NKC, FB], BF16)
        tmd3_b = ser.tile([P, NKC, FB], BF16)
        tq3_b = ser.tile([P, NKC, FB], BF16)
        x2row_b = rowsp.tile([P, RJ, N], BF16)
        g_x2 = []

        # inner combos (identity terms folded into the E/P evicts as c*Y)
        COMBOS = [(tp_b, 0.1 / 24.0, 0.1 / 120.0),
                  (tmd_b, 0.0625, 0.1 * 31.0 / 120.0),
                  (tmd3_b, 0.1 * 65.0 / 24.0, 0.1 * 211.0 / 120.0),
                  (tq3_b, 4.5, 3.375)]

        def x2_chunk(m):
            sl = slabp.tile([P, NKC, P], BF16, tag="slab", bufs=3, name="slab")
            nc.sync.dma_start(
                sl[:],
                adjmi_bf[:, m * P:(m + 1) * P].rearrange("(k p) n -> p k n", p=P))
            ps = psum.tile([P, FB], F32, tag="mm0", bufs=2, name="ps_mm0")
            for k in range(NKC):
                nc.tensor.matmul(ps[:], sl[:, k, :], xts_b[:, k, :],
                                 start=(k == 0), stop=(k == NKC - 1))
            nc.vector.tensor_copy(x2t_b[:, m, :], ps[:])
            for j in range(RJ):
                pe_t(x2row_b[:, j, m * P:(m + 1) * P],
                     x2t_b[:, m, j * P:(j + 1) * P])

        def x2_gather(h):
            ccin = dram.tile([RJ * (NKC // 2) * P, P], BF16, name=f"ccin_x2{h}")
            h0 = h * (NKC // 2) * P
            nt = NKC // 2
            for j in range(RJ):
                nc.sync.dma_start(
                    ccin[j * nt * P:(j + 1) * nt * P, :].rearrange(
                        "(t p) n -> p t n", p=P),
                    x2row_b[:, j, h0:h0 + nt * P].rearrange(
                        "p (t n) -> p t n", n=P))
            g = dram.tile([N_CORES * RJ * (NKC // 2) * P, P], BF16,
                          addr_space="Shared", name=f"g_x2{h}")
            nc.gpsimd.collective_compute(
                "AllGather", AL.bypass, replica_groups=LGROUP,
                ins=[ccin.opt()], outs=[g.opt()])
            g_x2.append(g)

        for m in range(NKC // 2):
            x2_chunk(m)
        x2_gather(0)

        # ---- w_mat pass (bf16): xr_b = 0.1(w diag(d) w^T - I) ----
        vr_raw = rowsp.tile([P, DKC, FBR], F32)
        nc.sync.dma_start(vr_raw[:],
                          w_colT.ap().rearrange("(k p) n -> p k n", p=P))
        vr = rowsp.tile([P, DKC, FBR], BF16)
        for k in range(DKC):
            nc.vector.tensor_scalar_mul(vr[:, k, :], vr_raw[:, k, :],
                                        d_sb[:, k:k + 1])
        xr_b = rowsp.tile([P, DKC, FBR], BF16)
        xr_row_b = rowsp.tile([P, D], BF16)
        for m in range(DKC):
            sl = slabp.tile([P, DKC, P], BF16, tag="rslab", bufs=2,
                            name="rslab")
            nc.sync.dma_start(
                sl[:],
                wT_full[:, m * P:(m + 1) * P].rearrange("(k p) n -> p k n", p=P))
            ps = psum.tile([P, FB], F32, tag="mm1", bufs=2, name="ps_mm1")
            for k in range(DKC):
                nc.tensor.matmul(ps[:, :FBR], sl[:, k, :], vr[:, k, :],
                                 start=(k == 0), stop=(k == DKC - 1))
            nc.vector.tensor_scalar_mul(xr_b[:, m, :], ps[:, :FBR], 0.1)
            nc.vector.scalar_tensor_tensor(xr_b[:, m, :], idn01[:],
                                           mR[:, m:m + 1], xr_b[:, m, :],
                                           AL.mult, AL.add)
            pe_t(xr_row_b[:, m * P:(m + 1) * P], xr_b[:, m, :])
        ccin_xr = dram.tile([DKC * P, P], BF16, name="ccin_xr")
        nc.sync.dma_start(
            ccin_xr[:].rearrange("(t p) n -> p t n", p=P), xr_row_b[:])
        g_xr = dram.tile([N_CORES * DKC * P, P], BF16, addr_space="Shared",
                         name="g_xr")
        nc.gpsimd.collective_compute(
            "AllGather", AL.bypass, replica_groups=LGROUP,
            ins=[ccin_xr.opt()], outs=[g_xr.opt()])

        for m in range(NKC // 2, NKC):
            x2_chunk(m)
        x2_gather(1)

        for m in range(NKC):
            for dst, c1, c2 in COMBOS:
                tmp = scrp.tile([P, FB], BF16, tag="combo", bufs=2, name="combo")
                nc.vector.tensor_scalar_mul(tmp[:], xt_b[:, m, :], c1)
                nc.vector.scalar_tensor_tensor(dst[:, m, :], x2t_b[:, m, :],
                                               c2, tmp[:], AL.mult, AL.add)

        # =========================================================
        # E/P pass, four series at once:
        #   elt = Y te + X            m2t = Y tp + .05X + .1I
        #   el2t = Y tq + 2X          mdt = Y tmd + .15X + .1I
        # =========================================================
        el3t_b = main.tile([P, NKC, FB], BF16)
        m2t_b = m2p.tile([P, NKC, FB], BF16)
        mdt_b = m2p.tile([P, NKC, FB], BF16)
        mdt3_b = m2p.tile([P, NKC, FB], BF16)

        def ep_chunk(m):
            h, mm = (0, m) if m < NKC // 2 else (1, m - NKC // 2)
            sl = slabp.tile([P, N_CORES, RJ, P], BF16, tag="slab", bufs=3,
                            name="slab")
            nc.sync.dma_start(
                sl[:],
                g_x2[h][:].rearrange("(c j t p) n -> p c j t n",
                                     c=N_CORES, j=RJ, p=P)[:, :, :, mm, :])
            ps0 = psum.tile([P, FB], F32, tag="mm0", bufs=2, name="ps_mm0")
            ps1 = psum.tile([P, FB], F32, tag="mm1", bufs=2, name="ps_mm1")
            ps2 = psum.tile([P, FB], F32, tag="mm2", bufs=1, name="ps_mm2")
            ps3 = psum.tile([P, FB], F32, tag="mm3", bufs=1, name="ps_mm3")
            for k in range(NKC):
                lt = sl[:, k // RJ, k % RJ, :]
                nc.tensor.matmul(ps0[:], lt, tp_b[:, k, :],
                                 start=(k == 0), stop=(k == NKC - 1))
                nc.tensor.matmul(ps1[:], lt, tmd_b[:, k, :],
                                 start=(k == 0), stop=(k == NKC - 1))
                nc.tensor.matmul(ps2[:], lt, tmd3_b[:, k, :],
                                 start=(k == 0), stop=(k == NKC - 1))
                nc.tensor.matmul(ps3[:], lt, tq3_b[:, k, :],
                                 start=(k == 0), stop=(k == NKC - 1))
            nc.vector.scalar_tensor_tensor(m2t_b[:, m, :], xt_b[:, m, :],
                                           0.05, ps0[:], AL.mult, AL.add)
            nc.vector.scalar_tensor_tensor(m2t_b[:, m, :], x2t_b[:, m, :],
                                           0.1 / 6.0, m2t_b[:, m, :],
                                           AL.mult, AL.add)
            nc.vector.scalar_tensor_tensor(mdt_b[:, m, :], xt_b[:, m, :],
                                           0.15, ps1[:], AL.mult, AL.add)
            nc.vector.scalar_tensor_tensor(mdt_b[:, m, :], x2t_b[:, m, :],
                                           0.7 / 6.0, mdt_b[:, m, :],
                                           AL.mult, AL.add)
            nc.vector.scalar_tensor_tensor(mdt3_b[:, m, :], xt_b[:, m, :],
                                           0.25, ps2[:], AL.mult, AL.add)
            nc.vector.scalar_tensor_tensor(mdt3_b[:, m, :], x2t_b[:, m, :],
                                           1.9 / 6.0, mdt3_b[:, m, :],
                                           AL.mult, AL.add)
            nc.vector.scalar_tensor_tensor(el3t_b[:, m, :], xt_b[:, m, :],
                                           3.0, ps3[:], AL.mult, AL.add)
            nc.vector.scalar_tensor_tensor(el3t_b[:, m, :], x2t_b[:, m, :],
                                           4.5, el3t_b[:, m, :],
                                           AL.mult, AL.add)
            for h2 in range(2):
                hs = slice(h2 * P, (h2 + 1) * P)
                msk = mL[:, 2 * m + h2:2 * m + h2 + 1]
                nc.vector.scalar_tensor_tensor(m2t_b[:, m, hs], id_m2[:], msk,
                                               m2t_b[:, m, hs], AL.mult, AL.add)
                nc.vector.scalar_tensor_tensor(mdt_b[:, m, hs], id_m2[:], msk,
                                               mdt_b[:, m, hs], AL.mult, AL.add)
                nc.vector.scalar_tensor_tensor(mdt3_b[:, m, hs], id_m2[:], msk,
                                               mdt3_b[:, m, hs], AL.mult, AL.add)

        for m in range(NKC // 2):
            ep_chunk(m)

        # ---- Xr^2 pass (between E/P halves; needs g_xr) ----
        xr2_b = rowsp.tile([P, DKC, FBR], BF16)
        for m in range(DKC):
            sl = slabp.tile([P, DKC, P], BF16, tag="rslab", bufs=2,
                            name="rslab")
            nc.sync.dma_start(
                sl[:],
                g_xr[:].rearrange("(c t p) n -> p c t n", c=N_CORES,
                                  p=P)[:, :, m, :])
            ps = psum.tile([P, FB], F32, tag="mm1", bufs=2, name="ps_mm1")
            for k in range(DKC):
                nc.tensor.matmul(ps[:, :FBR], sl[:, k, :], xr_b[:, k, :],
                                 start=(k == 0), stop=(k == DKC - 1))
            nc.vector.tensor_copy(xr2_b[:, m, :], ps[:, :FBR])

        for m in range(NKC // 2, NKC):
            ep_chunk(m)

        # ---- R-series: inner combos (no identity terms) ----
        tr_b = rowsp.tile([P, DKC, FBR], BF16, name="tr_b")
        tqr_b = rowsp.tile([P, DKC, FBR], BF16, name="tqr_b")
        tqr3_b = rowsp.tile([P, DKC, FBR], BF16, name="tqr3_b")
        RC = [(tr_b, 1.0 / 6.0, 1.0 / 24.0), (tqr_b, 4.0 / 3.0, 2.0 / 3.0),
              (tqr3_b, 4.5, 3.375)]
        for m in range(DKC):
            for dst, c1, c2 in RC:
                nc.vector.tensor_scalar_mul(dst[:, m, :], xr_b[:, m, :], c1)
                nc.vector.scalar_tensor_tensor(dst[:, m, :], xr2_b[:, m, :],
                                               c2, dst[:, m, :],
                                               AL.mult, AL.add)
        # u_i = Xr t_i + c_i Xr   (lhsT = gathered Xr)
        u1_b = rowsp.tile([P, DKC, FBR], BF16, name="u1_b")
        u2_b = rowsp.tile([P, DKC, FBR], BF16, name="u2_b")
        u3_b = rowsp.tile([P, DKC, FBR], BF16, name="u3_b")
        UC = [(u1_b, tr_b, 0.5), (u2_b, tqr_b, 2.0), (u3_b, tqr3_b, 4.5)]
        for m in range(DKC):
            sl = slabp.tile([P, DKC, P], BF16, tag="rslab", bufs=2,
                            name="rslab")
            nc.sync.dma_start(
                sl[:],
                g_xr[:].rearrange("(c t p) n -> p c t n", c=N_CORES,
                                  p=P)[:, :, m, :])
            pss = [psum.tile([P, FB], F32, tag=tg, bufs=(2 if tg == "mm1" else 1),
                             name=f"ps_{tg}")
                   for tg in ("mm1", "mm2", "mm3")]
            for k in range(DKC):
                for ps_, (_, rhs, _c) in zip(pss, UC):
                    nc.tensor.matmul(ps_[:, :FBR], sl[:, k, :], rhs[:, k, :],
                                     start=(k == 0), stop=(k == DKC - 1))
            for ps_, (dst, _rhs, ci) in zip(pss, UC):
                nc.vector.scalar_tensor_tensor(dst[:, m, :], xr_b[:, m, :],
                                               ci, ps_[:, :FBR],
                                               AL.mult, AL.add)
        # er_i = i*Xr + Xr u_i  (= e^{i Xr} - I)
        er_row_b = rowsp.tile([P, D], BF16)
        er2_row_b = rowsp.tile([P, D], BF16)
        er3_row_b = rowsp.tile([P, D], BF16)
        EC_ = [(er_row_b, u1_b, 1.0), (er2_row_b, u2_b, 2.0),
               (er3_row_b, u3_b, 3.0)]
        for m in range(DKC):
            sl = slabp.tile([P, DKC, P], BF16, tag="rslab", bufs=2,
                            name="rslab")
            nc.sync.dma_start(
                sl[:],
                g_xr[:].rearrange("(c t p) n -> p c t n", c=N_CORES,
                                  p=P)[:, :, m, :])
            pss = [psum.tile([P, FB], F32, tag=tg, bufs=(2 if tg == "mm1" else 1),
                             name=f"ps_{tg}")
                   for tg in ("mm1", "mm2", "mm3")]
            for k in range(DKC):
                for ps_, (_, rhs, _c) in zip(pss, EC_):
                    nc.tensor.matmul(ps_[:, :FBR], sl[:, k, :], rhs[:, k, :],
                                     start=(k == 0), stop=(k == DKC - 1))
            for ps_, (rowt, _rhs, ci) in zip(pss, EC_):
                erc = scrp.tile([P, FBR], BF16, tag="erc", bufs=3, name="erc")
                nc.vector.scalar_tensor_tensor(erc[:], xr_b[:, m, :], ci,
                                               ps_[:, :FBR], AL.mult, AL.add)
                pe_t(rowt[:, m * P:(m + 1) * P], erc[:])
        g_ers = []
        for nm, rowt in (("er", er_row_b), ("er2", er2_row_b),
                         ("er3", er3_row_b)):
            ccin = dram.tile([DKC * P, P], BF16, name=f"ccin_{nm}")
            nc.sync.dma_start(
                ccin[:].rearrange("(t p) n -> p t n", p=P), rowt[:])
            g = dram.tile([N_CORES * DKC * P, P], BF16, addr_space="Shared",
                          name=f"g_{nm}")
            nc.gpsimd.collective_compute(
                "AllGather", AL.bypass, replica_groups=LGROUP,
                ins=[ccin.opt()], outs=[g.opt()])
            g_ers.append(g)
        g_er, g_er2, g_er3 = g_ers

        rows_st.close()

        # ---- forcing: ft/f2t/f3t = (m2|md|md3 @ x0)^T  (fp32) ----
        ft = main.tile([P, DKC, FB], F32)
        f2t = main.tile([P, DKC, FB], F32)
        f2t_b = main.tile([P, DKC, FB], BF16)
        f3t = main.tile([P, DKC, FB], F32)
        f3t_b = main.tile([P, DKC, FB], BF16)
        for m in range(DKC):
            sl = slabp.tile([P, NKC, P], BF16, tag="slab", bufs=3, name="slab")
            nc.sync.dma_start(
                sl[:],
                x0_bf[:, m * P:(m + 1) * P].rearrange("(k p) n -> p k n", p=P))
            ps0 = psum.tile([P, FB], F32, tag="mm0", bufs=2, name="ps_mm0")
            ps1 = psum.tile([P, FB], F32, tag="mm1", bufs=2, name="ps_mm1")
            ps2 = psum.tile([P, FB], F32, tag="mm2", bufs=1, name="ps_mm2")
            for k in range(NKC):
                nc.tensor.matmul(ps0[:], sl[:, k, :], m2t_b[:, k, :],
                                 start=(k == 0), stop=(k == NKC - 1))
                nc.tensor.matmul(ps1[:], sl[:, k, :], mdt_b[:, k, :],
                                 start=(k == 0), stop=(k == NKC - 1))
                nc.tensor.matmul(ps2[:], sl[:, k, :], mdt3_b[:, k, :],
                                 start=(k == 0), stop=(k == NKC - 1))
            nc.vector.tensor_copy(ft[:, m, :], ps0[:])
            nc.vector.tensor_copy(f2t[:, m, :], ps1[:])
            nc.vector.tensor_copy(f2t_b[:, m, :], ps1[:])
            nc.vector.tensor_copy(f3t[:, m, :], ps2[:])
            nc.vector.tensor_copy(f3t_b[:, m, :], ps2[:])
        m2_st.close()
        ser_st.close()

        # ---- recurrence-phase pool ----
        rec = top.enter_context(tc.tile_pool(name="rec", bufs=1))
        er_sb = rec.tile([P, DKC, DKC, P], BF16)
        nc.sync.dma_start(
            er_sb[:],
            g_er[:].rearrange("(c t p) n -> p c t n", c=N_CORES, p=P))
        er2_sb = rec.tile([P, DKC, DKC, P], BF16)
        nc.sync.dma_start(
            er2_sb[:],
            g_er2[:].rearrange("(c t p) n -> p c t n", c=N_CORES, p=P))
        er3_sb = rec.tile([P, DKC, DKC, P], BF16)
        nc.sync.dma_start(
            er3_sb[:],
            g_er3[:].rearrange("(c t p) n -> p c t n", c=N_CORES, p=P))

        icp = [rec.tile([P, DKC, FB], F32, name=f"ict{i}") for i in range(2)]
        nc.sync.dma_start(
            icp[0][:], x_colT.ap().rearrange("(q p) n -> p q n", p=P))

        gt = rec.tile([P, DKC, FB], F32)

        # =========================================================
        # recurrence: step 0 single (elt/er/ft), steps 1..4 doubled
        # (el2t/er2/gt).  G computed between z1's V- and R-passes.
        # =========================================================
        ic_g = None
        for t in range(NTRIPLE):
            ict = icp[t % 2]
            icnt = icp[(t + 1) % 2]
            el_rhs = el3t_b
            er_lhs = er3_sb
            f_add = gt
            v = rec.tile([P, DKC, FB], F32, tag="v", bufs=1, name="v")
            v_b = rec.tile([P, DKC, FB], BF16, tag="v_b", bufs=1, name="v_b")
            for m in range(DKC):
                if t == 0:
                    sl = slabp.tile([P, NKC, P], BF16, tag="slab", bufs=3,
                                    name="slab")
                    nc.sync.dma_start(
                        sl[:],
                        x_bf[:, m * P:(m + 1) * P].rearrange(
                            "(k p) n -> p k n", p=P))
                    lt = lambda k: sl[:, k, :]
                else:
                    h, mm = (0, m) if m < DKC // 2 else (1, m - DKC // 2)
                    sl = slabp.tile([P, N_CORES, RJ, P], BF16,
                                    tag="slab", bufs=3, name="slab")
                    nc.sync.dma_start(
                        sl[:],
                        ic_g[h][:].rearrange(
                            "(c j t p) n -> p c j t n", c=N_CORES, j=RJ,
                            p=P)[:, :, :, mm, :])
                    lt = lambda k: sl[:, k // RJ, k % RJ, :]
                ps = psum.tile([P, FB], F32, tag="mm0", bufs=2, name="ps_mm0")
                for k in range(NKC):
                    nc.tensor.matmul(ps[:], lt(k), el_rhs[:, k, :],
                                     start=(k == 0), stop=(k == NKC - 1))
                nc.vector.scalar_tensor_tensor(v[:, m, :], ict[:, m, :],
                                               1.0, ps[:], AL.mult, AL.add)
                nc.vector.tensor_copy(v_b[:, m, :], v[:, m, :])

            if t == 0:
                # G3 = F + (F2 + F2@E_R) + (F3 + F3@E_R2)
                for m in range(DKC):
                    ps1 = psum.tile([P, FB], F32, tag="mm1", bufs=2,
                                    name="ps_mm1")
                    ps2 = psum.tile([P, FB], F32, tag="mm2", bufs=1,
                                    name="ps_mm2")
                    for k in range(DKC):
                        nc.tensor.matmul(ps1[:], er_sb[:, k, m, :],
                                         f2t_b[:, k, :],
                                         start=(k == 0), stop=(k == DKC - 1))
                        nc.tensor.matmul(ps2[:], er2_sb[:, k, m, :],
                                         f3t_b[:, k, :],
                                         start=(k == 0), stop=(k == DKC - 1))
                    nc.vector.scalar_tensor_tensor(gt[:, m, :], f2t[:, m, :],
                                                   1.0, ps1[:], AL.mult, AL.add)
                    nc.vector.scalar_tensor_tensor(gt[:, m, :], f3t[:, m, :],
                                                   1.0, gt[:, m, :],
                                                   AL.mult, AL.add)
                    nc.vector.scalar_tensor_tensor(gt[:, m, :], ps2[:],
                                                   1.0, gt[:, m, :],
                                                   AL.mult, AL.add)
                    nc.vector.scalar_tensor_tensor(gt[:, m, :], ft[:, m, :],
                                                   1.0, gt[:, m, :],
                                                   AL.mult, AL.add)

            icrow_b = rec.tile([P, RJ, D], BF16, tag="icrow", bufs=2,
                               name="icrow")
            for m in range(DKC):
                ps = psum.tile([P, FB], F32, tag="mm1", bufs=2, name="ps_mm1")
                for k in range(DKC):
                    nc.tensor.matmul(ps[:], er_lhs[:, k, m, :], v_b[:, k, :],
                                     start=(k == 0), stop=(k == DKC - 1))
                nc.vector.scalar_tensor_tensor(icnt[:, m, :], v[:, m, :],
                                               1.0, ps[:], AL.mult, AL.add)
                nc.vector.scalar_tensor_tensor(icnt[:, m, :], f_add[:, m, :],
                                               1.0, icnt[:, m, :],
                                               AL.mult, AL.add)
                if t < NTRIPLE - 1:
                    for j in range(RJ):
                        pe_t(icrow_b[:, j, m * P:(m + 1) * P],
                             icnt[:, m, j * P:(j + 1) * P])
                    if m == DKC // 2 - 1 or m == DKC - 1:
                        h = 0 if m < DKC // 2 else 1
                        h0 = h * (DKC // 2) * P
                        nq = DKC // 2
                        ccin = dram.tile([RJ * nq * P, P], BF16,
                                         tag="ccin_ic", name=f"ccin_ic{t}_{h}")
                        for j in range(RJ):
                            nc.sync.dma_start(
                                ccin[j * nq * P:(j + 1) * nq * P, :].rearrange(
                                    "(q p) n -> p q n", p=P),
                                icrow_b[:, j, h0:h0 + nq * P].rearrange(
                                    "p (q n) -> p q n", n=P))
                        g = dram.tile([N_CORES * RJ * nq * P, P],
                                      BF16, addr_space="Shared",
                                      name=f"g_ic{t}_{h}")
                        nc.gpsimd.collective_compute(
                            "AllGather", AL.bypass, replica_groups=LGROUP,
                            ins=[ccin.opt()], outs=[g.opt()])
                        if m == DKC // 2 - 1:
                            ic_g = [g]
                        else:
                            ic_g.append(g)

        # ---- output ----
        zrow = rec.tile([P, RJ, D], F32, name="zrow")
        icfin = icp[NTRIPLE % 2]
        for m in range(DKC):
            for j in range(RJ):
                pe_t(zrow[:, j, m * P:(m + 1) * P],
                     icfin[:, m, j * P:(j + 1) * P])
        for j in range(RJ):
            nc.sync.dma_start(z_loc[j * P:(j + 1) * P, :], zrow[:, j, :])

    nc.compile()
    return nc


_NC_CACHE = []


def _get_nc():
    if not _NC_CACHE:
        _NC_CACHE.append(build_nc())
    return _NC_CACHE[0]


def make_in_maps(inputs):
    x = np.ascontiguousarray(np.asarray(inputs["x"], dtype=np.float32))
    x0 = np.ascontiguousarray(np.asarray(inputs["x0"], dtype=np.float32))
    adj = np.ascontiguousarray(np.asarray(inputs["adj"], dtype=np.float32))
    alpha = np.ascontiguousarray(np.asarray(inputs["alpha_train"],
                                            dtype=np.float32))
    w = np.ascontiguousarray(np.asarray(inputs["w"], dtype=np.float32))
    d = np.ascontiguousarray(np.asarray(inputs["d"], dtype=np.float32))

    adjmi = adj.copy()
    np.fill_diagonal(adjmi, np.diagonal(adjmi) - 1.0)
    adjmi_b = adjmi.astype(BFNP)
    x_b = x.astype(BFNP)
    x0_b = x0.astype(BFNP)
    wT = np.ascontiguousarray(w.T.astype(BFNP))

    in_maps = []
    for c in range(N_CORES):
        r0 = c * RB
        f0 = c * FBR
        ml = np.zeros((P, NKC * 2), np.float32)
        ml[:, 2 * (2 * c)] = 1.0          # chunk 2c, half 0
        ml[:, 2 * (2 * c + 1) + 1] = 1.0  # chunk 2c+1, half 1
        mr = np.zeros((P, DKC), np.float32)
        mr[:, c] = 1.0
        in_maps.append({
            "adjmi_rows": np.ascontiguousarray(adjmi_b[r0:r0 + RB, :]),
            "alpha_blk": np.ascontiguousarray(alpha[r0:r0 + RB]),
            "x_colT": np.ascontiguousarray(x[r0:r0 + RB, :].T),
            "w_colT": np.ascontiguousarray(w[f0:f0 + FBR, :].T),
            "masksL": ml,
            "masksR": mr,
            "adjmi_bf": adjmi_b,
            "alpha_full": alpha,
            "x_bf": x_b,
            "x0_bf": x0_b,
            "wT_full": wT,
            "d_full": d,
        })
    return in_maps


def kernel(**inputs) -> np.ndarray:
    nc = _get_nc()
    in_maps = make_in_maps(inputs)
    res = run_bass_kernel_spmd(nc, in_maps, core_ids=list(range(N_CORES)))
    z = np.concatenate([res.results[c]["z_loc"] for c in range(N_CORES)], axis=0)
    return np.ascontiguousarray(z.astype(np.float32))


if __name__ == "__main__":
    rng = np.random.default_rng(0)
    ins = {
        "x": rng.standard_normal((N, D)).astype(np.float32),
        "x0": rng.standard_normal((N, D)).astype(np.float32),
        "adj": (rng.random((N, N)) / N).astype(np.float32),
        "alpha_train": rng.standard_normal((N,)).astype(np.float32),
        "w": (np.eye(D) + 0.02 * rng.standard_normal((D, D))).astype(np.float32),
        "d": rng.random((D,)).astype(np.float32),
    }
    out = kernel(**ins)
    print("kernel output:", out.shape, out.dtype, float(np.linalg.norm(out)))


# revision 20
# speedup vs baseline: 2.8973x; 1.0683x over previous
"""Trainium2 Bass kernel for the ETD1 ODE block (nn_ODEblockW_28922309771809).

Math (mirrors the jax reference; degree-4 Taylor, exact to ~1e-6 here since
||dt*A||_2 ~ 0.05 and ||dt*B||_2 ~ 0.16):
  s  = 0.05 * sigmoid(alpha);  X = diag(s)(adj - I);  Y = X^2
  Xr = 0.1((w*clip(d,0,1)) w^T - I);  Yr = Xr^2
The 9 recurrence steps  IC <- m1_L IC m1_R + F  (m1_L = e^X, m1_R = e^{Xr},
F = dt*phi1(X) x0) are regrouped as THREE triple steps
  IC <- m1_L^3 IC m1_R^3 + G3,   G3 = F + m1_L F m1_R + m1_L^2 F m1_R^2,
which needs only TWO inter-step IC AllGathers.  Every operator is a
polynomial in the same X, Y (resp. Xr, Yr), via e^{kX}-I and the phi1
identities  m1_L m2 = dt(2 phi1(2X) - phi1(X)),
            m1_L^2 m2 = dt(3 phi1(3X) - 2 phi1(2X)):
  E_L3 = e^{3X}-I = 3X + 4.5Y + Y(4.5X + 3.375Y)
  m2   = 0.1 I + .05X + (0.1/6)Y  + Y*0.1(X/24   + Y/120)
  md   = 0.1 I + .15X + (0.7/6)Y  + Y*0.1(5X/8   + 31Y/120)    # m1_L m2
  md3  = 0.1 I + .25X + (1.9/6)Y  + Y*0.1(65X/24 + 211Y/120)   # m1_L^2 m2
  E_R  = Xr  + Xr(Xr/2? no: Xr u1), u1 = Xr(X r/6+Yr/24) + Xr/2   (chained,
  E_R2 = 2Xr + Xr u2,  u2 = Xr(4Xr/3+2Yr/3)  + 2Xr               no Yr
  E_R3 = 3Xr + Xr u3,  u3 = Xr(4.5Xr+3.375Yr) + 4.5Xr            gather)
  F = m2 x0;  F2 = md x0;  F3 = md3 x0;  G3 = F + F2 + F2@E_R + F3 + F3@E_R2
  step: V = IC + E_L3@IC ; IC <- V + V@E_R3 + G3

Distribution over 8 cores (transposed-column-local): node dim sharded 256
rows/core, feature dim 128/core.  Full matrices needed as matmul lhsT come
from replicated DRAM inputs (adj-I, w^T, x, x0 - no collective) or from
bf16 tiled AllGathers (Y in 2 halves, Xr, E_R, E_R2, E_R3, and IC per step
in 2 halves).  The X^2 pass streams raw (adj-I) slabs and folds the
diag(s) row scaling into a pre-scaled rhs.  The four L-series come out of
ONE four-rhs matmul pass over gathered Y; identity terms of the inner
combos are folded into the evicts as c*Y adds (no masked-eye ops there).

Precision: quadratic+ series terms and all matmul inputs bf16; linear
terms and state accumulation fp32.  Measured ~2.9e-3 frob rel err vs the
fp32 reference (tolerance 2e-2).
"""

from contextlib import ExitStack

import numpy as np
import ml_dtypes

import concourse.bass as bass
import concourse.mybir as mybir
import concourse.tile as tile
from concourse import bacc
from concourse.bass_utils import run_bass_kernel_spmd
from concourse.masks import make_identity

F32 = mybir.dt.float32
F32R = mybir.dt.float32r
BF16 = mybir.dt.bfloat16
AL = mybir.AluOpType

N_CORES = 8
P = 128
N = 2048          # nodes
D = 1024          # features
RB = 256          # node row-block per core
FB = 256          # L-side col width (= RB)
FBR = 128         # feature block width per core
NKC = N // P      # 16
DKC = D // P      # 8
RJ = RB // P      # 2
NTRIPLE = 3       # 9 steps = 3 tripled

LGROUP = [list(range(N_CORES))]
BFNP = ml_dtypes.bfloat16


def build_nc():
    nc = bacc.Bacc("TRN2", target_bir_lowering=False, debug=False,
                   num_devices=N_CORES)

    # ---- per-core inputs ----
    adjmi_rows = nc.dram_tensor("adjmi_rows", [RB, N], BF16, kind="ExternalInput")
    alpha_blk = nc.dram_tensor("alpha_blk", [RB], F32, kind="ExternalInput")
    x_colT = nc.dram_tensor("x_colT", [D, RB], F32, kind="ExternalInput")
    w_colT = nc.dram_tensor("w_colT", [D, FBR], F32, kind="ExternalInput")
    masksL = nc.dram_tensor("masksL", [P, NKC * 2], F32, kind="ExternalInput")
    masksR = nc.dram_tensor("masksR", [P, DKC], F32, kind="ExternalInput")
    # ---- replicated inputs (same array on every core) ----
    adjmi_bf = nc.dram_tensor("adjmi_bf", [N, N], BF16, kind="ExternalInput")
    alpha_full = nc.dram_tensor("alpha_full", [N], F32, kind="ExternalInput")
    x_bf = nc.dram_tensor("x_bf", [N, D], BF16, kind="ExternalInput")
    x0_bf = nc.dram_tensor("x0_bf", [N, D], BF16, kind="ExternalInput")
    wT_full = nc.dram_tensor("wT_full", [D, D], BF16, kind="ExternalInput")
    d_full = nc.dram_tensor("d_full", [D], F32, kind="ExternalInput")
    z_loc = nc.dram_tensor("z_loc", [RB, D], F32, kind="ExternalOutput")

    with tile.TileContext(nc) as tc, ExitStack() as top:
        const = top.enter_context(tc.tile_pool(name="const", bufs=1))
        dram = top.enter_context(tc.tile_pool(name="dram", bufs=1, space="DRAM"))
        psum = top.enter_context(tc.tile_pool(name="psum", bufs=2, space="PSUM"))
        slabp = top.enter_context(tc.tile_pool(name="slabp", bufs=1))
        scrp = top.enter_context(tc.tile_pool(name="scrp", bufs=1))
        main = top.enter_context(tc.tile_pool(name="main", bufs=1))

        ident = const.tile([P, P], F32)
        make_identity(nc, ident)
        ident_b = const.tile([P, P], BF16)
        nc.vector.tensor_copy(ident_b[:], ident[:])
        # scaled identity tiles for masked diagonal adds
        id_m2 = const.tile([P, P], BF16)       # 0.1 I   (m2/md/md3 evicts)
        nc.vector.tensor_scalar_mul(id_m2[:], ident[:], 0.1)
        idn01 = const.tile([P, P], BF16)       # -0.1 I  (Xr)
        nc.vector.tensor_scalar_mul(idn01[:], ident[:], -0.1)

        mL = const.tile([P, NKC * 2], F32)
        nc.sync.dma_start(mL[:], masksL[:])
        mR = const.tile([P, DKC], F32)
        nc.sync.dma_start(mR[:], masksR[:])

        def pe_t(dst_slice, src_slice):
            """dst[128,128] = src[128,128].T via PE transpose."""
            if src_slice.dtype == F32R:
                src_slice = src_slice.bitcast(F32)
            fp32_in = src_slice.dtype == F32
            ps = psum.tile([P, P], F32 if fp32_in else BF16, tag="tr",
                           bufs=2, name="ps_tr")
            nc.tensor.transpose(ps[:], src_slice,
                                ident[:] if fp32_in else ident_b[:])
            nc.vector.tensor_copy(dst_slice, ps[:])

        # =========================================================
        # scales
        # =========================================================
        s_sb = const.tile([P, RJ], F32)
        nc.sync.dma_start(s_sb[:], alpha_blk.ap().rearrange("(j p) -> p j", p=P))
        nc.scalar.activation(s_sb[:], s_sb[:],
                             mybir.ActivationFunctionType.Sigmoid)
        nc.vector.tensor_scalar_mul(s_sb[:], s_sb[:], 0.05)

        s_full = const.tile([P, NKC], F32)
        nc.sync.dma_start(s_full[:], alpha_full.ap().rearrange("(k p) -> p k", p=P))
        nc.scalar.activation(s_full[:], s_full[:],
                             mybir.ActivationFunctionType.Sigmoid)
        nc.vector.tensor_scalar_mul(s_full[:], s_full[:], 0.05)

        d_sb = const.tile([P, DKC], F32)
        nc.sync.dma_start(d_sb[:], d_full.ap().rearrange("(q p) -> p q", p=P))
        nc.vector.tensor_scalar(d_sb[:], d_sb[:], 0.0, 1.0, AL.max, AL.min)

        # =========================================================
        # phase pools (stack: ser -> m2p -> rows; close rows, m2p, ser)
        # =========================================================
        ser_st = ExitStack()
        ser = ser_st.enter_context(tc.tile_pool(name="ph_ser", bufs=1))
        m2_st = ExitStack()
        m2p = m2_st.enter_context(tc.tile_pool(name="ph_m2", bufs=1))
        rows_st = ExitStack()
        rowsp = rows_st.enter_context(tc.tile_pool(name="ph_rows", bufs=1))

        # ---- xt_b = X^T[:, own 256 cols]; xts_b = diag(s) X^T ----
        xt_b = ser.tile([P, NKC, FB], BF16)
        xts_b = ser.tile([P, NKC, FB], BF16)
        with tc.tile_pool(name="ph_x", bufs=1) as ph_x:
            xrow_b = ph_x.tile([P, RJ, N], BF16)
            for j in range(RJ):
                raw = scrp.tile([P, N], BF16, tag="adj_in", bufs=1, name="adj_in")
                nc.sync.dma_start(raw[:], adjmi_rows[j * P:(j + 1) * P, :])
                nc.vector.tensor_scalar_mul(xrow_b[:, j, :], raw[:],
                                            s_sb[:, j:j + 1])
            for k in range(NKC):
                for j in range(RJ):
                    pe_t(xt_b[:, k, j * P:(j + 1) * P],
                         xrow_b[:, j, k * P:(k + 1) * P])
        for k in range(NKC):
            nc.vector.tensor_scalar_mul(xts_b[:, k, :], xt_b[:, k, :],
                                        s_full[:, k:k + 1])

        # =========================================================
        # X^2 pass (raw adj-I slabs, pre-scaled rhs).  Per chunk the psum
        # (= Y^T chunk, fp32) feeds the four series combos directly.
        # =========================================================
        x2t_b = ser.tile([P, NKC, FB], BF16)
        tq3_b = ser.tile([P, NKC, FB], BF16)
        x2row_b = rowsp.tile([P, RJ, N], BF16)
        g_x2 = []

        # only E_L3 needs the quartic terms; the forcing series (m2, md,
        # md3) truncate after Y (error ~1e-5 of F) and are DVE-only combos
        COMBOS = [(tq3_b, 4.5, 3.375)]

        def x2_chunk(m):
            sl = slabp.tile([P, NKC, P], BF16, tag="slab", bufs=3, name="slab")
            nc.sync.dma_start(
                sl[:],
                adjmi_bf[:, m * P:(m + 1) * P].rearrange("(k p) n -> p k n", p=P))
            ps = psum.tile([P, FB], F32, tag="mm0", bufs=2, name="ps_mm0")
            for k in range(NKC):
                nc.tensor.matmul(ps[:], sl[:, k, :], xts_b[:, k, :],
                                 start=(k == 0), stop=(k == NKC - 1))
            nc.vector.tensor_copy(x2t_b[:, m, :], ps[:])
            for j in range(RJ):
                pe_t(x2row_b[:, j, m * P:(m + 1) * P],
                     x2t_b[:, m, j * P:(j + 1) * P])

        def x2_gather(h):
            ccin = dram.tile([RJ * (NKC // 2) * P, P], BF16, name=f"ccin_x2{h}")
            h0 = h * (NKC // 2) * P
            nt = NKC // 2
            for j in range(RJ):
                nc.sync.dma_start(
                    ccin[j * nt * P:(j + 1) * nt * P, :].rearrange(
                        "(t p) n -> p t n", p=P),
                    x2row_b[:, j, h0:h0 + nt * P].rearrange(
                        "p (t n) -> p t n", n=P))
            g = dram.tile([N_CORES * RJ * (NKC // 2) * P, P], BF16,
                          addr_space="Shared", name=f"g_x2{h}")
            nc.gpsimd.collective_compute(
                "AllGather", AL.bypass, replica_groups=LGROUP,
                ins=[ccin.opt()], outs=[g.opt()])
            g_x2.append(g)

        for m in range(NKC // 2):
            x2_chunk(m)
        x2_gather(0)

        # ---- w_mat pass (bf16): xr_b = 0.1(w diag(d) w^T - I) ----
        vr_raw = rowsp.tile([P, DKC, FBR], F32)
        nc.sync.dma_start(vr_raw[:],
                          w_colT.ap().rearrange("(k p) n -> p k n", p=P))
        vr = rowsp.tile([P, DKC, FBR], BF16)
        for k in range(DKC):
            nc.vector.tensor_scalar_mul(vr[:, k, :], vr_raw[:, k, :],
                                        d_sb[:, k:k + 1])
        xr_b = rowsp.tile([P, DKC, FBR], BF16)
        xr_row_b = rowsp.tile([P, D], BF16)
        for m in range(DKC):
            sl = slabp.tile([P, DKC, P], BF16, tag="rslab", bufs=2,
                            name="rslab")
            nc.sync.dma_start(
                sl[:],
                wT_full[:, m * P:(m + 1) * P].rearrange("(k p) n -> p k n", p=P))
            ps = psum.tile([P, FB], F32, tag="mm1", bufs=2, name="ps_mm1")
            for k in range(DKC):
                nc.tensor.matmul(ps[:, :FBR], sl[:, k, :], vr[:, k, :],
                                 start=(k == 0), stop=(k == DKC - 1))
            nc.vector.tensor_scalar_mul(xr_b[:, m, :], ps[:, :FBR], 0.1)
            nc.vector.scalar_tensor_tensor(xr_b[:, m, :], idn01[:],
                                           mR[:, m:m + 1], xr_b[:, m, :],
                                           AL.mult, AL.add)
            pe_t(xr_row_b[:, m * P:(m + 1) * P], xr_b[:, m, :])
        ccin_xr = dram.tile([DKC * P, P], BF16, name="ccin_xr")
        nc.sync.dma_start(
            ccin_xr[:].rearrange("(t p) n -> p t n", p=P), xr_row_b[:])
        g_xr = dram.tile([N_CORES * DKC * P, P], BF16, addr_space="Shared",
                         name="g_xr")
        nc.gpsimd.collective_compute(
            "AllGather", AL.bypass, replica_groups=LGROUP,
            ins=[ccin_xr.opt()], outs=[g_xr.opt()])

        for m in range(NKC // 2, NKC):
            x2_chunk(m)
        x2_gather(1)

        FSER = [(None, 0.05, 0.1 / 6.0), (None, 0.15, 0.7 / 6.0),
                (None, 0.25, 1.9 / 6.0)]

        def build_fser(m):
            for dst, c1, c2 in COMBOS + FSER:
                tmp = scrp.tile([P, FB], BF16, tag="combo", bufs=2, name="combo")
                nc.vector.tensor_scalar_mul(tmp[:], xt_b[:, m, :], c1)
                nc.vector.scalar_tensor_tensor(dst[:, m, :], x2t_b[:, m, :],
                                               c2, tmp[:], AL.mult, AL.add)
                if dst in (m2t_b, mdt_b, mdt3_b):
                    for h in range(2):
                        hs = slice(h * P, (h + 1) * P)
                        nc.vector.scalar_tensor_tensor(
                            dst[:, m, hs], id_m2[:],
                            mL[:, 2 * m + h:2 * m + h + 1],
                            dst[:, m, hs], AL.mult, AL.add)

        # =========================================================
        # E/P pass, four series at once:
        #   elt = Y te + X            m2t = Y tp + .05X + .1I
        #   el2t = Y tq + 2X          mdt = Y tmd + .15X + .1I
        # =========================================================
        el3t_b = main.tile([P, NKC, FB], BF16)
        m2t_b = m2p.tile([P, NKC, FB], BF16)
        mdt_b = m2p.tile([P, NKC, FB], BF16)
        mdt3_b = m2p.tile([P, NKC, FB], BF16)
        FSER[0] = (m2t_b, 0.05, 0.1 / 6.0)
        FSER[1] = (mdt_b, 0.15, 0.7 / 6.0)
        FSER[2] = (mdt3_b, 0.25, 1.9 / 6.0)
        for m in range(NKC):
            build_fser(m)

        def ep_chunk(m):
            h, mm = (0, m) if m < NKC // 2 else (1, m - NKC // 2)
            sl = slabp.tile([P, N_CORES, RJ, P], BF16, tag="slab", bufs=3,
                            name="slab")
            nc.sync.dma_start(
                sl[:],
                g_x2[h][:].rearrange("(c j t p) n -> p c j t n",
                                     c=N_CORES, j=RJ, p=P)[:, :, :, mm, :])
            ps3 = psum.tile([P, FB], F32, tag="mm3", bufs=1, name="ps_mm3")
            for k in range(NKC):
                lt = sl[:, k // RJ, k % RJ, :]
                nc.tensor.matmul(ps3[:], lt, tq3_b[:, k, :],
                                 start=(k == 0), stop=(k == NKC - 1))
            nc.vector.scalar_tensor_tensor(el3t_b[:, m, :], xt_b[:, m, :],
                                           3.0, ps3[:], AL.mult, AL.add)
            nc.vector.scalar_tensor_tensor(el3t_b[:, m, :], x2t_b[:, m, :],
                                           4.5, el3t_b[:, m, :],
                                           AL.mult, AL.add)

        for m in range(NKC // 2):
            ep_chunk(m)

        # ---- Xr^2 pass (between E/P halves; needs g_xr) ----
        xr2_b = rowsp.tile([P, DKC, FBR], BF16)
        for m in range(DKC):
            sl = slabp.tile([P, DKC, P], BF16, tag="rslab", bufs=2,
                            name="rslab")
            nc.sync.dma_start(
                sl[:],
                g_xr[:].rearrange("(c t p) n -> p c t n", c=N_CORES,
                                  p=P)[:, :, m, :])
            ps = psum.tile([P, FB], F32, tag="mm1", bufs=2, name="ps_mm1")
            for k in range(DKC):
                nc.tensor.matmul(ps[:, :FBR], sl[:, k, :], xr_b[:, k, :],
                                 start=(k == 0), stop=(k == DKC - 1))
            nc.vector.tensor_copy(xr2_b[:, m, :], ps[:, :FBR])

        # ---- forcing: ft/f2t/f3t = (m2|md|md3 @ x0)^T  (fp32) ----
        ft = main.tile([P, DKC, FB], F32)
        f2t = main.tile([P, DKC, FB], F32)
        f2t_b = main.tile([P, DKC, FB], BF16)
        f3t = main.tile([P, DKC, FB], F32)
        f3t_b = main.tile([P, DKC, FB], BF16)
        for m in range(DKC):
            sl = slabp.tile([P, NKC, P], BF16, tag="slab", bufs=3, name="slab")
            nc.sync.dma_start(
                sl[:],
                x0_bf[:, m * P:(m + 1) * P].rearrange("(k p) n -> p k n", p=P))
            ps0 = psum.tile([P, FB], F32, tag="mm0", bufs=2, name="ps_mm0")
            ps1 = psum.tile([P, FB], F32, tag="mm1", bufs=2, name="ps_mm1")
            ps2 = psum.tile([P, FB], F32, tag="mm2", bufs=1, name="ps_mm2")
            for k in range(NKC):
                nc.tensor.matmul(ps0[:], sl[:, k, :], m2t_b[:, k, :],
                                 start=(k == 0), stop=(k == NKC - 1))
                nc.tensor.matmul(ps1[:], sl[:, k, :], mdt_b[:, k, :],
                                 start=(k == 0), stop=(k == NKC - 1))
                nc.tensor.matmul(ps2[:], sl[:, k, :], mdt3_b[:, k, :],
                                 start=(k == 0), stop=(k == NKC - 1))
            nc.vector.tensor_copy(ft[:, m, :], ps0[:])
            nc.vector.tensor_copy(f2t[:, m, :], ps1[:])
            nc.vector.tensor_copy(f2t_b[:, m, :], ps1[:])
            nc.vector.tensor_copy(f3t[:, m, :], ps2[:])
            nc.vector.tensor_copy(f3t_b[:, m, :], ps2[:])

        for m in range(NKC // 2, NKC):
            ep_chunk(m)

        # ---- R-series: inner combos (no identity terms) ----
        tr_b = rowsp.tile([P, DKC, FBR], BF16, name="tr_b")
        tqr_b = rowsp.tile([P, DKC, FBR], BF16, name="tqr_b")
        tqr3_b = rowsp.tile([P, DKC, FBR], BF16, name="tqr3_b")
        RC = [(tr_b, 1.0 / 6.0, 1.0 / 24.0), (tqr_b, 4.0 / 3.0, 2.0 / 3.0),
              (tqr3_b, 4.5, 3.375)]
        for m in range(DKC):
            for dst, c1, c2 in RC:
                nc.vector.tensor_scalar_mul(dst[:, m, :], xr_b[:, m, :], c1)
                nc.vector.scalar_tensor_tensor(dst[:, m, :], xr2_b[:, m, :],
                                               c2, dst[:, m, :],
                                               AL.mult, AL.add)
        # u_i = Xr t_i + c_i Xr   (lhsT = gathered Xr)
        u1_b = rowsp.tile([P, DKC, FBR], BF16, name="u1_b")
        u2_b = rowsp.tile([P, DKC, FBR], BF16, name="u2_b")
        u3_b = rowsp.tile([P, DKC, FBR], BF16, name="u3_b")
        UC = [(u1_b, tr_b, 0.5), (u2_b, tqr_b, 2.0), (u3_b, tqr3_b, 4.5)]
        for m in range(DKC):
            sl = slabp.tile([P, DKC, P], BF16, tag="rslab", bufs=2,
                            name="rslab")
            nc.sync.dma_start(
                sl[:],
                g_xr[:].rearrange("(c t p) n -> p c t n", c=N_CORES,
                                  p=P)[:, :, m, :])
            pss = [psum.tile([P, FB], F32, tag=tg, bufs=(2 if tg == "mm1" else 1),
                             name=f"ps_{tg}")
                   for tg in ("mm1", "mm2", "mm3")]
            for k in range(DKC):
                for ps_, (_, rhs, _c) in zip(pss, UC):
                    nc.tensor.matmul(ps_[:, :FBR], sl[:, k, :], rhs[:, k, :],
                                     start=(k == 0), stop=(k == DKC - 1))
            for ps_, (dst, _rhs, ci) in zip(pss, UC):
                nc.vector.scalar_tensor_tensor(dst[:, m, :], xr_b[:, m, :],
                                               ci, ps_[:, :FBR],
                                               AL.mult, AL.add)
        # er_i = i*Xr + Xr u_i  (= e^{i Xr} - I)
        er_row_b = rowsp.tile([P, D], BF16)
        er2_row_b = rowsp.tile([P, D], BF16)
        er3_row_b = rowsp.tile([P, D], BF16)
        EC_ = [(er_row_b, u1_b, 1.0), (er2_row_b, u2_b, 2.0),
               (er3_row_b, u3_b, 3.0)]
        for m in range(DKC):
            sl = slabp.tile([P, DKC, P], BF16, tag="rslab", bufs=2,
                            name="rslab")
            nc.sync.dma_start(
                sl[:],
                g_xr[:].rearrange("(c t p) n -> p c t n", c=N_CORES,
                                  p=P)[:, :, m, :])
            pss = [psum.tile([P, FB], F32, tag=tg, bufs=(2 if tg == "mm1" else 1),
                             name=f"ps_{tg}")
                   for tg in ("mm1", "mm2", "mm3")]
            for k in range(DKC):
                for ps_, (_, rhs, _c) in zip(pss, EC_):
                    nc.tensor.matmul(ps_[:, :FBR], sl[:, k, :], rhs[:, k, :],
                                     start=(k == 0), stop=(k == DKC - 1))
            for ps_, (rowt, _rhs, ci) in zip(pss, EC_):
                erc = scrp.tile([P, FBR], BF16, tag="erc", bufs=3, name="erc")
                nc.vector.scalar_tensor_tensor(erc[:], xr_b[:, m, :], ci,
                                               ps_[:, :FBR], AL.mult, AL.add)
                pe_t(rowt[:, m * P:(m + 1) * P], erc[:])
        g_ers = []
        for nm, rowt in (("er3", er3_row_b), ("er", er_row_b),
                         ("er2", er2_row_b)):
            ccin = dram.tile([DKC * P, P], BF16, name=f"ccin_{nm}")
            nc.sync.dma_start(
                ccin[:].rearrange("(t p) n -> p t n", p=P), rowt[:])
            g = dram.tile([N_CORES * DKC * P, P], BF16, addr_space="Shared",
                          name=f"g_{nm}")
            nc.gpsimd.collective_compute(
                "AllGather", AL.bypass, replica_groups=LGROUP,
                ins=[ccin.opt()], outs=[g.opt()])
            g_ers.append(g)
        g_er3, g_er, g_er2 = g_ers

        rows_st.close()

        m2_st.close()
        ser_st.close()

        # ---- recurrence-phase pool ----
        rec = top.enter_context(tc.tile_pool(name="rec", bufs=1))
        er3_sb = rec.tile([P, DKC, DKC, P], BF16)
        nc.sync.dma_start(
            er3_sb[:],
            g_er3[:].rearrange("(c t p) n -> p c t n", c=N_CORES, p=P))
        er_sb = rec.tile([P, DKC, DKC, P], BF16)
        nc.sync.dma_start(
            er_sb[:],
            g_er[:].rearrange("(c t p) n -> p c t n", c=N_CORES, p=P))
        er2_sb = rec.tile([P, DKC, DKC, P], BF16)
        nc.sync.dma_start(
            er2_sb[:],
            g_er2[:].rearrange("(c t p) n -> p c t n", c=N_CORES, p=P))

        icp = [rec.tile([P, DKC, FB], F32, name=f"ict{i}") for i in range(2)]
        nc.sync.dma_start(
            icp[0][:], x_colT.ap().rearrange("(q p) n -> p q n", p=P))

        gt = rec.tile([P, DKC, FB], F32)

        # =========================================================
        # recurrence: step 0 single (elt/er/ft), steps 1..4 doubled
        # (el2t/er2/gt).  G computed between z1's V- and R-passes.
        # =========================================================
        ic_g = None
        for t in range(NTRIPLE):
            ict = icp[t % 2]
            icnt = icp[(t + 1) % 2]
            el_rhs = el3t_b
            er_lhs = er3_sb
            f_add = gt
            v = rec.tile([P, DKC, FB], F32, tag="v", bufs=1, name="v")
            v_b = rec.tile([P, DKC, FB], BF16, tag="v_b", bufs=1, name="v_b")
            for m in range(DKC):
                if t == 0:
                    sl = slabp.tile([P, NKC, P], BF16, tag="slab", bufs=3,
                                    name="slab")
                    nc.sync.dma_start(
                        sl[:],
                        x_bf[:, m * P:(m + 1) * P].rearrange(
                            "(k p) n -> p k n", p=P))
                    lt = lambda k: sl[:, k, :]
                else:
                    h, mm = (0, m) if m < DKC // 2 else (1, m - DKC // 2)
                    sl = slabp.tile([P, N_CORES, RJ, P], BF16,
                                    tag="slab", bufs=3, name="slab")
                    nc.sync.dma_start(
                        sl[:],
                        ic_g[h][:].rearrange(
                            "(c j t p) n -> p c j t n", c=N_CORES, j=RJ,
                            p=P)[:, :, :, mm, :])
                    lt = lambda k: sl[:, k // RJ, k % RJ, :]
                ps = psum.tile([P, FB], F32, tag="mm0", bufs=2, name="ps_mm0")
                for k in range(NKC):
                    nc.tensor.matmul(ps[:], lt(k), el_rhs[:, k, :],
                                     start=(k == 0), stop=(k == NKC - 1))
                nc.vector.scalar_tensor_tensor(v[:, m, :], ict[:, m, :],
                                               1.0, ps[:], AL.mult, AL.add)
                nc.vector.tensor_copy(v_b[:, m, :], v[:, m, :])

            if t == 0:
                # G3 = F + (F2 + F2@E_R) + (F3 + F3@E_R2)
                for m in range(DKC):
                    ps1 = psum.tile([P, FB], F32, tag="mm1", bufs=2,
                                    name="ps_mm1")
                    ps2 = psum.tile([P, FB], F32, tag="mm2", bufs=1,
                                    name="ps_mm2")
                    for k in range(DKC):
                        nc.tensor.matmul(ps1[:], er_sb[:, k, m, :],
                                         f2t_b[:, k, :],
                                         start=(k == 0), stop=(k == DKC - 1))
                        nc.tensor.matmul(ps2[:], er2_sb[:, k, m, :],
                                         f3t_b[:, k, :],
                                         start=(k == 0), stop=(k == DKC - 1))
                    nc.vector.scalar_tensor_tensor(gt[:, m, :], f2t[:, m, :],
                                                   1.0, ps1[:], AL.mult, AL.add)
                    nc.vector.scalar_tensor_tensor(gt[:, m, :], f3t[:, m, :],
                                                   1.0, gt[:, m, :],
                                                   AL.mult, AL.add)
                    nc.vector.scalar_tensor_tensor(gt[:, m, :], ps2[:],
                                                   1.0, gt[:, m, :],
                                                   AL.mult, AL.add)
                    nc.vector.scalar_tensor_tensor(gt[:, m, :], ft[:, m, :],
                                                   1.0, gt[:, m, :],
                                                   AL.mult, AL.add)

            icrow_b = rec.tile([P, RJ, D], BF16, tag="icrow", bufs=2,
                               name="icrow")
            for m in range(DKC):
                ps = psum.tile([P, FB], F32, tag="mm1", bufs=2, name="ps_mm1")
                for k in range(DKC):
                    nc.tensor.matmul(ps[:], er_lhs[:, k, m, :], v_b[:, k, :],
                                     start=(k == 0), stop=(k == DKC - 1))
                nc.vector.scalar_tensor_tensor(icnt[:, m, :], v[:, m, :],
                                               1.0, ps[:], AL.mult, AL.add)
                nc.vector.scalar_tensor_tensor(icnt[:, m, :], f_add[:, m, :],
                                               1.0, icnt[:, m, :],
                                               AL.mult, AL.add)
                if t < NTRIPLE - 1:
                    for j in range(RJ):
                        pe_t(icrow_b[:, j, m * P:(m + 1) * P],
                             icnt[:, m, j * P:(j + 1) * P])
                    if m == DKC // 2 - 1 or m == DKC - 1:
                        h = 0 if m < DKC // 2 else 1
                        h0 = h * (DKC // 2) * P
                        nq = DKC // 2
                        ccin = dram.tile([RJ * nq * P, P], BF16,
                                         tag="ccin_ic", name=f"ccin_ic{t}_{h}")
                        for j in range(RJ):
                            nc.sync.dma_start(
                                ccin[j * nq * P:(j + 1) * nq * P, :].rearrange(
                                    "(q p) n -> p q n", p=P),
                                icrow_b[:, j, h0:h0 + nq * P].rearrange(
                                    "p (q n) -> p q n", n=P))
                        g = dram.tile([N_CORES * RJ * nq * P, P],
                                      BF16, addr_space="Shared",
                                      name=f"g_ic{t}_{h}")
                        nc.gpsimd.collective_compute(
                            "AllGather", AL.bypass, replica_groups=LGROUP,
                            ins=[ccin.opt()], outs=[g.opt()])
                        if m == DKC // 2 - 1:
                            ic_g = [g]
                        else:
                            ic_g.append(g)

        # ---- output ----
        zrow = rec.tile([P, RJ, D], F32, name="zrow")
        icfin = icp[NTRIPLE % 2]
        for m in range(DKC):
            for j in range(RJ):
                pe_t(zrow[:, j, m * P:(m + 1) * P],
                     icfin[:, m, j * P:(j + 1) * P])
        for j in range(RJ):
            nc.sync.dma_start(z_loc[j * P:(j + 1) * P, :], zrow[:, j, :])

    nc.compile()
    return nc


_NC_CACHE = []


def _get_nc():
    if not _NC_CACHE:
        _NC_CACHE.append(build_nc())
    return _NC_CACHE[0]


def make_in_maps(inputs):
    x = np.ascontiguousarray(np.asarray(inputs["x"], dtype=np.float32))
    x0 = np.ascontiguousarray(np.asarray(inputs["x0"], dtype=np.float32))
    adj = np.ascontiguousarray(np.asarray(inputs["adj"], dtype=np.float32))
    alpha = np.ascontiguousarray(np.asarray(inputs["alpha_train"],
                                            dtype=np.float32))
    w = np.ascontiguousarray(np.asarray(inputs["w"], dtype=np.float32))
    d = np.ascontiguousarray(np.asarray(inputs["d"], dtype=np.float32))

    adjmi = adj.copy()
    np.fill_diagonal(adjmi, np.diagonal(adjmi) - 1.0)
    adjmi_b = adjmi.astype(BFNP)
    x_b = x.astype(BFNP)
    x0_b = x0.astype(BFNP)
    wT = np.ascontiguousarray(w.T.astype(BFNP))

    in_maps = []
    for c in range(N_CORES):
        r0 = c * RB
        f0 = c * FBR
        ml = np.zeros((P, NKC * 2), np.float32)
        ml[:, 2 * (2 * c)] = 1.0          # chunk 2c, half 0
        ml[:, 2 * (2 * c + 1) + 1] = 1.0  # chunk 2c+1, half 1
        mr = np.zeros((P, DKC), np.float32)
        mr[:, c] = 1.0
        in_maps.append({
            "adjmi_rows": np.ascontiguousarray(adjmi_b[r0:r0 + RB, :]),
            "alpha_blk": np.ascontiguousarray(alpha[r0:r0 + RB]),
            "x_colT": np.ascontiguousarray(x[r0:r0 + RB, :].T),
            "w_colT": np.ascontiguousarray(w[f0:f0 + FBR, :].T),
            "masksL": ml,
            "masksR": mr,
            "adjmi_bf": adjmi_b,
            "alpha_full": alpha,
            "x_bf": x_b,
            "x0_bf": x0_b,
            "wT_full": wT,
            "d_full": d,
        })
    return in_maps


def kernel(**inputs) -> np.ndarray:
    nc = _get_nc()
    in_maps = make_in_maps(inputs)
    res = run_bass_kernel_spmd(nc, in_maps, core_ids=list(range(N_CORES)))
    z = np.concatenate([res.results[c]["z_loc"] for c in range(N_CORES)], axis=0)
    return np.ascontiguousarray(z.astype(np.float32))


if __name__ == "__main__":
    rng = np.random.default_rng(0)
    ins = {
        "x": rng.standard_normal((N, D)).astype(np.float32),
        "x0": rng.standard_normal((N, D)).astype(np.float32),
        "adj": (rng.random((N, N)) / N).astype(np.float32),
        "alpha_train": rng.standard_normal((N,)).astype(np.float32),
        "w": (np.eye(D) + 0.02 * rng.standard_normal((D, D))).astype(np.float32),
        "d": rng.random((D,)).astype(np.float32),
    }
    out = kernel(**ins)
    print("kernel output:", out.shape, out.dtype, float(np.linalg.norm(out)))


# revision 22
# speedup vs baseline: 2.9398x; 1.0147x over previous
"""Trainium2 Bass kernel for the ETD1 ODE block (nn_ODEblockW_28922309771809).

Math (mirrors the jax reference; degree-4 Taylor, exact to ~1e-6 here since
||dt*A||_2 ~ 0.05 and ||dt*B||_2 ~ 0.16):
  s  = 0.05 * sigmoid(alpha);  X = diag(s)(adj - I);  Y = X^2
  Xr = 0.1((w*clip(d,0,1)) w^T - I);  Yr = Xr^2
The 9 recurrence steps  IC <- m1_L IC m1_R + F  (m1_L = e^X, m1_R = e^{Xr},
F = dt*phi1(X) x0) are regrouped as THREE triple steps
  IC <- m1_L^3 IC m1_R^3 + G3,   G3 = F + m1_L F m1_R + m1_L^2 F m1_R^2,
which needs only TWO inter-step IC AllGathers.  Every operator is a
polynomial in the same X, Y (resp. Xr, Yr), via e^{kX}-I and the phi1
identities  m1_L m2 = dt(2 phi1(2X) - phi1(X)),
            m1_L^2 m2 = dt(3 phi1(3X) - 2 phi1(2X)):
  E_L3 = e^{3X}-I = 3X + 4.5Y + Y(4.5X + 3.375Y)
  m2   = 0.1 I + .05X + (0.1/6)Y  + Y*0.1(X/24   + Y/120)
  md   = 0.1 I + .15X + (0.7/6)Y  + Y*0.1(5X/8   + 31Y/120)    # m1_L m2
  md3  = 0.1 I + .25X + (1.9/6)Y  + Y*0.1(65X/24 + 211Y/120)   # m1_L^2 m2
  E_R  = Xr  + Xr(Xr/2? no: Xr u1), u1 = Xr(X r/6+Yr/24) + Xr/2   (chained,
  E_R2 = 2Xr + Xr u2,  u2 = Xr(4Xr/3+2Yr/3)  + 2Xr               no Yr
  E_R3 = 3Xr + Xr u3,  u3 = Xr(4.5Xr+3.375Yr) + 4.5Xr            gather)
  F = m2 x0;  F2 = md x0;  F3 = md3 x0;  G3 = F + F2 + F2@E_R + F3 + F3@E_R2
  step: V = IC + E_L3@IC ; IC <- V + V@E_R3 + G3

Distribution over 8 cores (transposed-column-local): node dim sharded 256
rows/core, feature dim 128/core.  Full matrices needed as matmul lhsT come
from replicated DRAM inputs (adj-I, w^T, x, x0 - no collective) or from
bf16 tiled AllGathers (Y in 2 halves, Xr, E_R, E_R2, E_R3, and IC per step
in 2 halves).  The X^2 pass streams raw (adj-I) slabs and folds the
diag(s) row scaling into a pre-scaled rhs.  The four L-series come out of
ONE four-rhs matmul pass over gathered Y; identity terms of the inner
combos are folded into the evicts as c*Y adds (no masked-eye ops there).

Precision: quadratic+ series terms and all matmul inputs bf16; linear
terms and state accumulation fp32.  Measured ~2.9e-3 frob rel err vs the
fp32 reference (tolerance 2e-2).
"""

from contextlib import ExitStack

import numpy as np
import ml_dtypes

import concourse.bass as bass
import concourse.mybir as mybir
import concourse.tile as tile
from concourse import bacc
from concourse.bass_utils import run_bass_kernel_spmd
from concourse.masks import make_identity

F32 = mybir.dt.float32
F32R = mybir.dt.float32r
BF16 = mybir.dt.bfloat16
AL = mybir.AluOpType

N_CORES = 8
P = 128
N = 2048          # nodes
D = 1024          # features
RB = 256          # node row-block per core
FB = 256          # L-side col width (= RB)
FBR = 128         # feature block width per core
NKC = N // P      # 16
DKC = D // P      # 8
RJ = RB // P      # 2
NTRIPLE = 3       # 9 steps = 3 tripled

LGROUP = [list(range(N_CORES))]
BFNP = ml_dtypes.bfloat16


def build_nc():
    nc = bacc.Bacc("TRN2", target_bir_lowering=False, debug=False,
                   num_devices=N_CORES)

    # ---- per-core inputs ----
    adjmi_rows = nc.dram_tensor("adjmi_rows", [RB, N], BF16, kind="ExternalInput")
    alpha_blk = nc.dram_tensor("alpha_blk", [RB], F32, kind="ExternalInput")
    x_colT = nc.dram_tensor("x_colT", [D, RB], F32, kind="ExternalInput")
    x0_colT = nc.dram_tensor("x0_colT", [D, RB], F32, kind="ExternalInput")
    w_colT = nc.dram_tensor("w_colT", [D, FBR], F32, kind="ExternalInput")
    masksL = nc.dram_tensor("masksL", [P, NKC * 2], F32, kind="ExternalInput")
    masksR = nc.dram_tensor("masksR", [P, DKC], F32, kind="ExternalInput")
    # ---- replicated inputs (same array on every core) ----
    adjmi_bf = nc.dram_tensor("adjmi_bf", [N, N], BF16, kind="ExternalInput")
    alpha_full = nc.dram_tensor("alpha_full", [N], F32, kind="ExternalInput")
    x_bf = nc.dram_tensor("x_bf", [N, D], BF16, kind="ExternalInput")
    x0_bf = nc.dram_tensor("x0_bf", [N, D], BF16, kind="ExternalInput")
    wT_full = nc.dram_tensor("wT_full", [D, D], BF16, kind="ExternalInput")
    d_full = nc.dram_tensor("d_full", [D], F32, kind="ExternalInput")
    z_loc = nc.dram_tensor("z_loc", [RB, D], F32, kind="ExternalOutput")

    with tile.TileContext(nc) as tc, ExitStack() as top:
        const = top.enter_context(tc.tile_pool(name="const", bufs=1))
        dram = top.enter_context(tc.tile_pool(name="dram", bufs=1, space="DRAM"))
        psum = top.enter_context(tc.tile_pool(name="psum", bufs=2, space="PSUM"))
        slabp = top.enter_context(tc.tile_pool(name="slabp", bufs=1))
        scrp = top.enter_context(tc.tile_pool(name="scrp", bufs=1))
        main = top.enter_context(tc.tile_pool(name="main", bufs=1))

        ident = const.tile([P, P], F32)
        make_identity(nc, ident)
        ident_b = const.tile([P, P], BF16)
        nc.vector.tensor_copy(ident_b[:], ident[:])
        # scaled identity tiles for masked diagonal adds
        id_m2 = const.tile([P, P], BF16)       # 0.1 I   (m2/md/md3 evicts)
        nc.vector.tensor_scalar_mul(id_m2[:], ident[:], 0.1)
        idn01 = const.tile([P, P], BF16)       # -0.1 I  (Xr)
        nc.vector.tensor_scalar_mul(idn01[:], ident[:], -0.1)

        mL = const.tile([P, NKC * 2], F32)
        nc.sync.dma_start(mL[:], masksL[:])
        mR = const.tile([P, DKC], F32)
        nc.sync.dma_start(mR[:], masksR[:])

        def pe_t(dst_slice, src_slice):
            """dst[128,128] = src[128,128].T via PE transpose."""
            if src_slice.dtype == F32R:
                src_slice = src_slice.bitcast(F32)
            fp32_in = src_slice.dtype == F32
            ps = psum.tile([P, P], F32 if fp32_in else BF16, tag="tr",
                           bufs=2, name="ps_tr")
            nc.tensor.transpose(ps[:], src_slice,
                                ident[:] if fp32_in else ident_b[:])
            nc.vector.tensor_copy(dst_slice, ps[:])

        # =========================================================
        # scales
        # =========================================================
        s_sb = const.tile([P, RJ], F32)
        nc.sync.dma_start(s_sb[:], alpha_blk.ap().rearrange("(j p) -> p j", p=P))
        nc.scalar.activation(s_sb[:], s_sb[:],
                             mybir.ActivationFunctionType.Sigmoid)
        nc.vector.tensor_scalar_mul(s_sb[:], s_sb[:], 0.05)

        s_full = const.tile([P, NKC], F32)
        nc.sync.dma_start(s_full[:], alpha_full.ap().rearrange("(k p) -> p k", p=P))
        nc.scalar.activation(s_full[:], s_full[:],
                             mybir.ActivationFunctionType.Sigmoid)
        nc.vector.tensor_scalar_mul(s_full[:], s_full[:], 0.05)

        d_sb = const.tile([P, DKC], F32)
        nc.sync.dma_start(d_sb[:], d_full.ap().rearrange("(q p) -> p q", p=P))
        nc.vector.tensor_scalar(d_sb[:], d_sb[:], 0.0, 1.0, AL.max, AL.min)

        # =========================================================
        # phase pools (stack: ser -> m2p -> rows; close rows, m2p, ser)
        # =========================================================
        ser_st = ExitStack()
        ser = ser_st.enter_context(tc.tile_pool(name="ph_ser", bufs=1))
        m2_st = ExitStack()
        m2p = m2_st.enter_context(tc.tile_pool(name="ph_m2", bufs=1))
        rows_st = ExitStack()
        rowsp = rows_st.enter_context(tc.tile_pool(name="ph_rows", bufs=1))

        # ---- xt_b = X^T[:, own 256 cols]; xts_b = diag(s) X^T ----
        xt_b = ser.tile([P, NKC, FB], BF16)
        xts_b = ser.tile([P, NKC, FB], BF16)
        with tc.tile_pool(name="ph_x", bufs=1) as ph_x:
            xrow_b = ph_x.tile([P, RJ, N], BF16)
            for j in range(RJ):
                raw = scrp.tile([P, N], BF16, tag="adj_in", bufs=1, name="adj_in")
                nc.sync.dma_start(raw[:], adjmi_rows[j * P:(j + 1) * P, :])
                nc.vector.tensor_scalar_mul(xrow_b[:, j, :], raw[:],
                                            s_sb[:, j:j + 1])
            for k in range(NKC):
                for j in range(RJ):
                    pe_t(xt_b[:, k, j * P:(j + 1) * P],
                         xrow_b[:, j, k * P:(k + 1) * P])
        for k in range(NKC):
            nc.vector.tensor_scalar_mul(xts_b[:, k, :], xt_b[:, k, :],
                                        s_full[:, k:k + 1])

        # =========================================================
        # X^2 pass (raw adj-I slabs, pre-scaled rhs).  Per chunk the psum
        # (= Y^T chunk, fp32) feeds the four series combos directly.
        # =========================================================
        x2t_b = ser.tile([P, NKC, FB], BF16)
        tq3_b = ser.tile([P, NKC, FB], BF16)
        x2row_b = rowsp.tile([P, RJ, N], BF16)
        g_x2 = []

        # only E_L3 needs the quartic terms; the forcing series (m2, md,
        # md3) truncate after Y (error ~1e-5 of F) and are DVE-only combos
        COMBOS = [(tq3_b, 4.5, 3.375)]

        def x2_chunk(m):
            sl = slabp.tile([P, NKC, P], BF16, tag="slab", bufs=3, name="slab")
            nc.sync.dma_start(
                sl[:],
                adjmi_bf[:, m * P:(m + 1) * P].rearrange("(k p) n -> p k n", p=P))
            ps = psum.tile([P, FB], F32, tag="mm0", bufs=2, name="ps_mm0")
            for k in range(NKC):
                nc.tensor.matmul(ps[:], sl[:, k, :], xts_b[:, k, :],
                                 start=(k == 0), stop=(k == NKC - 1))
            nc.vector.tensor_copy(x2t_b[:, m, :], ps[:])
            for j in range(RJ):
                pe_t(x2row_b[:, j, m * P:(m + 1) * P],
                     x2t_b[:, m, j * P:(j + 1) * P])

        def x2_gather(h):
            ccin = dram.tile([RJ * (NKC // 2) * P, P], BF16, name=f"ccin_x2{h}")
            h0 = h * (NKC // 2) * P
            nt = NKC // 2
            for j in range(RJ):
                nc.sync.dma_start(
                    ccin[j * nt * P:(j + 1) * nt * P, :].rearrange(
                        "(t p) n -> p t n", p=P),
                    x2row_b[:, j, h0:h0 + nt * P].rearrange(
                        "p (t n) -> p t n", n=P))
            g = dram.tile([N_CORES * RJ * (NKC // 2) * P, P], BF16,
                          addr_space="Shared", name=f"g_x2{h}")
            nc.gpsimd.collective_compute(
                "AllGather", AL.bypass, replica_groups=LGROUP,
                ins=[ccin.opt()], outs=[g.opt()])
            g_x2.append(g)

        for m in range(NKC // 2):
            x2_chunk(m)
        x2_gather(0)

        # ---- w_mat pass (bf16): xr_b = 0.1(w diag(d) w^T - I) ----
        vr_raw = rowsp.tile([P, DKC, FBR], F32)
        nc.sync.dma_start(vr_raw[:],
                          w_colT.ap().rearrange("(k p) n -> p k n", p=P))
        vr = rowsp.tile([P, DKC, FBR], BF16)
        for k in range(DKC):
            nc.vector.tensor_scalar_mul(vr[:, k, :], vr_raw[:, k, :],
                                        d_sb[:, k:k + 1])
        xr_b = rowsp.tile([P, DKC, FBR], BF16)
        xr_row_b = rowsp.tile([P, D], BF16)
        for m in range(DKC):
            sl = slabp.tile([P, DKC, P], BF16, tag="rslab", bufs=2,
                            name="rslab")
            nc.sync.dma_start(
                sl[:],
                wT_full[:, m * P:(m + 1) * P].rearrange("(k p) n -> p k n", p=P))
            ps = psum.tile([P, FB], F32, tag="mm1", bufs=2, name="ps_mm1")
            for k in range(DKC):
                nc.tensor.matmul(ps[:, :FBR], sl[:, k, :], vr[:, k, :],
                                 start=(k == 0), stop=(k == DKC - 1))
            nc.vector.tensor_scalar_mul(xr_b[:, m, :], ps[:, :FBR], 0.1)
            nc.vector.scalar_tensor_tensor(xr_b[:, m, :], idn01[:],
                                           mR[:, m:m + 1], xr_b[:, m, :],
                                           AL.mult, AL.add)
            pe_t(xr_row_b[:, m * P:(m + 1) * P], xr_b[:, m, :])
        ccin_xr = dram.tile([DKC * P, P], BF16, name="ccin_xr")
        nc.sync.dma_start(
            ccin_xr[:].rearrange("(t p) n -> p t n", p=P), xr_row_b[:])
        g_xr = dram.tile([N_CORES * DKC * P, P], BF16, addr_space="Shared",
                         name="g_xr")
        nc.gpsimd.collective_compute(
            "AllGather", AL.bypass, replica_groups=LGROUP,
            ins=[ccin_xr.opt()], outs=[g_xr.opt()])

        for m in range(NKC // 2, NKC):
            x2_chunk(m)
        x2_gather(1)

        # =========================================================
        # E/P pass, four series at once:
        #   elt = Y te + X            m2t = Y tp + .05X + .1I
        #   el2t = Y tq + 2X          mdt = Y tmd + .15X + .1I
        # =========================================================
        el3t_b = main.tile([P, NKC, FB], BF16)
        for m in range(NKC):
            for dst, c1, c2 in COMBOS:
                tmp = scrp.tile([P, FB], BF16, tag="combo", bufs=2, name="combo")
                nc.vector.tensor_scalar_mul(tmp[:], xt_b[:, m, :], c1)
                nc.vector.scalar_tensor_tensor(dst[:, m, :], x2t_b[:, m, :],
                                               c2, tmp[:], AL.mult, AL.add)

        def ep_chunk(m):
            h, mm = (0, m) if m < NKC // 2 else (1, m - NKC // 2)
            sl = slabp.tile([P, N_CORES, RJ, P], BF16, tag="slab", bufs=3,
                            name="slab")
            nc.sync.dma_start(
                sl[:],
                g_x2[h][:].rearrange("(c j t p) n -> p c j t n",
                                     c=N_CORES, j=RJ, p=P)[:, :, :, mm, :])
            ps3 = psum.tile([P, FB], F32, tag="mm3", bufs=1, name="ps_mm3")
            for k in range(NKC):
                lt = sl[:, k // RJ, k % RJ, :]
                nc.tensor.matmul(ps3[:], lt, tq3_b[:, k, :],
                                 start=(k == 0), stop=(k == NKC - 1))
            nc.vector.scalar_tensor_tensor(el3t_b[:, m, :], xt_b[:, m, :],
                                           3.0, ps3[:], AL.mult, AL.add)
            nc.vector.scalar_tensor_tensor(el3t_b[:, m, :], x2t_b[:, m, :],
                                           4.5, el3t_b[:, m, :],
                                           AL.mult, AL.add)

        for m in range(NKC // 2):
            ep_chunk(m)

        # ---- Xr^2 pass (between E/P halves; needs g_xr) ----
        xr2_b = rowsp.tile([P, DKC, FBR], BF16)
        for m in range(DKC):
            sl = slabp.tile([P, DKC, P], BF16, tag="rslab", bufs=2,
                            name="rslab")
            nc.sync.dma_start(
                sl[:],
                g_xr[:].rearrange("(c t p) n -> p c t n", c=N_CORES,
                                  p=P)[:, :, m, :])
            ps = psum.tile([P, FB], F32, tag="mm1", bufs=2, name="ps_mm1")
            for k in range(DKC):
                nc.tensor.matmul(ps[:, :FBR], sl[:, k, :], xr_b[:, k, :],
                                 start=(k == 0), stop=(k == DKC - 1))
            nc.vector.tensor_copy(xr2_b[:, m, :], ps[:, :FBR])

        # ---- forcing: F_i = 0.1 x0 + a_i (X@x0) + b_i (Y@x0) ----
        x0t = main.tile([P, DKC, FB], F32)
        nc.sync.dma_start(
            x0t[:], x0_colT.ap().rearrange("(q p) n -> p q n", p=P))
        ft = main.tile([P, DKC, FB], F32)
        f2t = main.tile([P, DKC, FB], F32)
        f2t_b = main.tile([P, DKC, FB], BF16)
        f3t = main.tile([P, DKC, FB], F32)
        f3t_b = main.tile([P, DKC, FB], BF16)
        FCO = [(ft, None, 0.05, 0.1 / 6.0), (f2t, None, 0.15, 0.7 / 6.0),
               (f3t, None, 0.25, 1.9 / 6.0)]
        for m in range(DKC):
            sl = slabp.tile([P, NKC, P], BF16, tag="slab", bufs=3, name="slab")
            nc.sync.dma_start(
                sl[:],
                x0_bf[:, m * P:(m + 1) * P].rearrange("(k p) n -> p k n", p=P))
            psA = psum.tile([P, FB], F32, tag="mm0", bufs=2, name="ps_mm0")
            psB = psum.tile([P, FB], F32, tag="mm1", bufs=2, name="ps_mm1")
            for k in range(NKC):
                nc.tensor.matmul(psA[:], sl[:, k, :], xt_b[:, k, :],
                                 start=(k == 0), stop=(k == NKC - 1))
                nc.tensor.matmul(psB[:], sl[:, k, :], x2t_b[:, k, :],
                                 start=(k == 0), stop=(k == NKC - 1))
            for dst, _, a, b in FCO:
                nc.vector.tensor_scalar_mul(dst[:, m, :], psA[:], a)
                nc.vector.scalar_tensor_tensor(dst[:, m, :], psB[:], b,
                                               dst[:, m, :], AL.mult, AL.add)
                nc.vector.scalar_tensor_tensor(dst[:, m, :], x0t[:, m, :],
                                               0.1, dst[:, m, :],
                                               AL.mult, AL.add)
            nc.vector.tensor_copy(f2t_b[:, m, :], f2t[:, m, :])
            nc.vector.tensor_copy(f3t_b[:, m, :], f3t[:, m, :])

        for m in range(NKC // 2, NKC):
            ep_chunk(m)

        # ---- R-series: inner combos (no identity terms) ----
        tr_b = rowsp.tile([P, DKC, FBR], BF16, name="tr_b")
        tqr_b = rowsp.tile([P, DKC, FBR], BF16, name="tqr_b")
        tqr3_b = rowsp.tile([P, DKC, FBR], BF16, name="tqr3_b")
        RC = [(tr_b, 1.0 / 6.0, 1.0 / 24.0), (tqr_b, 4.0 / 3.0, 2.0 / 3.0),
              (tqr3_b, 4.5, 3.375)]
        for m in range(DKC):
            for dst, c1, c2 in RC:
                nc.vector.tensor_scalar_mul(dst[:, m, :], xr_b[:, m, :], c1)
                nc.vector.scalar_tensor_tensor(dst[:, m, :], xr2_b[:, m, :],
                                               c2, dst[:, m, :],
                                               AL.mult, AL.add)
        # u_i = Xr t_i + c_i Xr   (lhsT = gathered Xr)
        u1_b = rowsp.tile([P, DKC, FBR], BF16, name="u1_b")
        u2_b = rowsp.tile([P, DKC, FBR], BF16, name="u2_b")
        u3_b = rowsp.tile([P, DKC, FBR], BF16, name="u3_b")
        UC = [(u1_b, tr_b, 0.5), (u2_b, tqr_b, 2.0), (u3_b, tqr3_b, 4.5)]
        for m in range(DKC):
            sl = slabp.tile([P, DKC, P], BF16, tag="rslab", bufs=2,
                            name="rslab")
            nc.sync.dma_start(
                sl[:],
                g_xr[:].rearrange("(c t p) n -> p c t n", c=N_CORES,
                                  p=P)[:, :, m, :])
            pss = [psum.tile([P, FB], F32, tag=tg, bufs=(2 if tg == "mm1" else 1),
                             name=f"ps_{tg}")
                   for tg in ("mm1", "mm2", "mm3")]
            for k in range(DKC):
                for ps_, (_, rhs, _c) in zip(pss, UC):
                    nc.tensor.matmul(ps_[:, :FBR], sl[:, k, :], rhs[:, k, :],
                                     start=(k == 0), stop=(k == DKC - 1))
            for ps_, (dst, _rhs, ci) in zip(pss, UC):
                nc.vector.scalar_tensor_tensor(dst[:, m, :], xr_b[:, m, :],
                                               ci, ps_[:, :FBR],
                                               AL.mult, AL.add)
        # er_i = i*Xr + Xr u_i  (= e^{i Xr} - I)
        er_row_b = rowsp.tile([P, D], BF16)
        er2_row_b = rowsp.tile([P, D], BF16)
        er3_row_b = rowsp.tile([P, D], BF16)
        EC_ = [(er_row_b, u1_b, 1.0), (er2_row_b, u2_b, 2.0),
               (er3_row_b, u3_b, 3.0)]
        for m in range(DKC):
            sl = slabp.tile([P, DKC, P], BF16, tag="rslab", bufs=2,
                            name="rslab")
            nc.sync.dma_start(
                sl[:],
                g_xr[:].rearrange("(c t p) n -> p c t n", c=N_CORES,
                                  p=P)[:, :, m, :])
            pss = [psum.tile([P, FB], F32, tag=tg, bufs=(2 if tg == "mm1" else 1),
                             name=f"ps_{tg}")
                   for tg in ("mm1", "mm2", "mm3")]
            for k in range(DKC):
                for ps_, (_, rhs, _c) in zip(pss, EC_):
                    nc.tensor.matmul(ps_[:, :FBR], sl[:, k, :], rhs[:, k, :],
                                     start=(k == 0), stop=(k == DKC - 1))
            for ps_, (rowt, _rhs, ci) in zip(pss, EC_):
                erc = scrp.tile([P, FBR], BF16, tag="erc", bufs=3, name="erc")
                nc.vector.scalar_tensor_tensor(erc[:], xr_b[:, m, :], ci,
                                               ps_[:, :FBR], AL.mult, AL.add)
                pe_t(rowt[:, m * P:(m + 1) * P], erc[:])
        ccin_er = dram.tile([3 * DKC * P, P], BF16, name="ccin_er")
        for gi, rowt in enumerate((er3_row_b, er_row_b, er2_row_b)):
            nc.sync.dma_start(
                ccin_er[gi * DKC * P:(gi + 1) * DKC * P, :].rearrange(
                    "(t p) n -> p t n", p=P), rowt[:])
        g_erall = dram.tile([N_CORES * 3 * DKC * P, P], BF16,
                            addr_space="Shared", name="g_erall")
        nc.gpsimd.collective_compute(
            "AllGather", AL.bypass, replica_groups=LGROUP,
            ins=[ccin_er.opt()], outs=[g_erall.opt()])

        rows_st.close()

        m2_st.close()
        ser_st.close()

        # ---- recurrence-phase pool ----
        rec = top.enter_context(tc.tile_pool(name="rec", bufs=1))
        er3_sb = rec.tile([P, DKC, DKC, P], BF16)
        er_sb = rec.tile([P, DKC, DKC, P], BF16)
        er2_sb = rec.tile([P, DKC, DKC, P], BF16)
        for gi, dst in enumerate((er3_sb, er_sb, er2_sb)):
            for c in range(N_CORES):
                nc.sync.dma_start(
                    dst[:, c, :, :],
                    g_erall[:].rearrange("(c g t p) n -> p c g t n",
                                         c=N_CORES, g=3, p=P)[:, c, gi, :, :])

        icp = [rec.tile([P, DKC, FB], F32, name=f"ict{i}") for i in range(2)]
        nc.sync.dma_start(
            icp[0][:], x_colT.ap().rearrange("(q p) n -> p q n", p=P))

        gt = rec.tile([P, DKC, FB], F32)

        # =========================================================
        # recurrence: step 0 single (elt/er/ft), steps 1..4 doubled
        # (el2t/er2/gt).  G computed between z1's V- and R-passes.
        # =========================================================
        ic_g = None
        for t in range(NTRIPLE):
            ict = icp[t % 2]
            icnt = icp[(t + 1) % 2]
            el_rhs = el3t_b
            er_lhs = er3_sb
            f_add = gt
            v = rec.tile([P, DKC, FB], F32, tag="v", bufs=1, name="v")
            v_b = rec.tile([P, DKC, FB], BF16, tag="v_b", bufs=1, name="v_b")
            for m in range(DKC):
                if t == 0:
                    sl = slabp.tile([P, NKC, P], BF16, tag="slab", bufs=3,
                                    name="slab")
                    nc.sync.dma_start(
                        sl[:],
                        x_bf[:, m * P:(m + 1) * P].rearrange(
                            "(k p) n -> p k n", p=P))
                    lt = lambda k: sl[:, k, :]
                else:
                    h, mm = (0, m) if m < DKC // 2 else (1, m - DKC // 2)
                    sl = slabp.tile([P, N_CORES, RJ, P], BF16,
                                    tag="slab", bufs=3, name="slab")
                    nc.sync.dma_start(
                        sl[:],
                        ic_g[h][:].rearrange(
                            "(c j t p) n -> p c j t n", c=N_CORES, j=RJ,
                            p=P)[:, :, :, mm, :])
                    lt = lambda k: sl[:, k // RJ, k % RJ, :]
                ps = psum.tile([P, FB], F32, tag="mm0", bufs=2, name="ps_mm0")
                for k in range(NKC):
                    nc.tensor.matmul(ps[:], lt(k), el_rhs[:, k, :],
                                     start=(k == 0), stop=(k == NKC - 1))
                nc.vector.scalar_tensor_tensor(v[:, m, :], ict[:, m, :],
                                               1.0, ps[:], AL.mult, AL.add)
                nc.vector.tensor_copy(v_b[:, m, :], v[:, m, :])

            if t == 0:
                # G3 = F + (F2 + F2@E_R) + (F3 + F3@E_R2)
                for m in range(DKC):
                    ps1 = psum.tile([P, FB], F32, tag="mm1", bufs=2,
                                    name="ps_mm1")
                    ps2 = psum.tile([P, FB], F32, tag="mm2", bufs=1,
                                    name="ps_mm2")
                    for k in range(DKC):
                        nc.tensor.matmul(ps1[:], er_sb[:, k, m, :],
                                         f2t_b[:, k, :],
                                         start=(k == 0), stop=(k == DKC - 1))
                        nc.tensor.matmul(ps2[:], er2_sb[:, k, m, :],
                                         f3t_b[:, k, :],
                                         start=(k == 0), stop=(k == DKC - 1))
                    nc.vector.scalar_tensor_tensor(gt[:, m, :], f2t[:, m, :],
                                                   1.0, ps1[:], AL.mult, AL.add)
                    nc.vector.scalar_tensor_tensor(gt[:, m, :], f3t[:, m, :],
                                                   1.0, gt[:, m, :],
                                                   AL.mult, AL.add)
                    nc.vector.scalar_tensor_tensor(gt[:, m, :], ps2[:],
                                                   1.0, gt[:, m, :],
                                                   AL.mult, AL.add)
                    nc.vector.scalar_tensor_tensor(gt[:, m, :], ft[:, m, :],
                                                   1.0, gt[:, m, :],
                                                   AL.mult, AL.add)

            icrow_b = rec.tile([P, RJ, D], BF16, tag="icrow", bufs=2,
                               name="icrow")
            for m in range(DKC):
                ps = psum.tile([P, FB], F32, tag="mm1", bufs=2, name="ps_mm1")
                for k in range(DKC):
                    nc.tensor.matmul(ps[:], er_lhs[:, k, m, :], v_b[:, k, :],
                                     start=(k == 0), stop=(k == DKC - 1))
                nc.vector.scalar_tensor_tensor(icnt[:, m, :], v[:, m, :],
                                               1.0, ps[:], AL.mult, AL.add)
                nc.vector.scalar_tensor_tensor(icnt[:, m, :], f_add[:, m, :],
                                               1.0, icnt[:, m, :],
                                               AL.mult, AL.add)
                if t < NTRIPLE - 1:
                    for j in range(RJ):
                        pe_t(icrow_b[:, j, m * P:(m + 1) * P],
                             icnt[:, m, j * P:(j + 1) * P])
                    if m == DKC // 2 - 1 or m == DKC - 1:
                        h = 0 if m < DKC // 2 else 1
                        h0 = h * (DKC // 2) * P
                        nq = DKC // 2
                        ccin = dram.tile([RJ * nq * P, P], BF16,
                                         tag="ccin_ic", name=f"ccin_ic{t}_{h}")
                        for j in range(RJ):
                            nc.sync.dma_start(
                                ccin[j * nq * P:(j + 1) * nq * P, :].rearrange(
                                    "(q p) n -> p q n", p=P),
                                icrow_b[:, j, h0:h0 + nq * P].rearrange(
                                    "p (q n) -> p q n", n=P))
                        g = dram.tile([N_CORES * RJ * nq * P, P],
                                      BF16, addr_space="Shared",
                                      name=f"g_ic{t}_{h}")
                        nc.gpsimd.collective_compute(
                            "AllGather", AL.bypass, replica_groups=LGROUP,
                            ins=[ccin.opt()], outs=[g.opt()])
                        if m == DKC // 2 - 1:
                            ic_g = [g]
                        else:
                            ic_g.append(g)

        # ---- output ----
        zrow = rec.tile([P, RJ, D], F32, name="zrow")
        icfin = icp[NTRIPLE % 2]
        for m in range(DKC):
            for j in range(RJ):
                pe_t(zrow[:, j, m * P:(m + 1) * P],
                     icfin[:, m, j * P:(j + 1) * P])
        for j in range(RJ):
            nc.sync.dma_start(z_loc[j * P:(j + 1) * P, :], zrow[:, j, :])

    nc.compile()
    return nc


_NC_CACHE = []


def _get_nc():
    if not _NC_CACHE:
        _NC_CACHE.append(build_nc())
    return _NC_CACHE[0]


def make_in_maps(inputs):
    x = np.ascontiguousarray(np.asarray(inputs["x"], dtype=np.float32))
    x0 = np.ascontiguousarray(np.asarray(inputs["x0"], dtype=np.float32))
    adj = np.ascontiguousarray(np.asarray(inputs["adj"], dtype=np.float32))
    alpha = np.ascontiguousarray(np.asarray(inputs["alpha_train"],
                                            dtype=np.float32))
    w = np.ascontiguousarray(np.asarray(inputs["w"], dtype=np.float32))
    d = np.ascontiguousarray(np.asarray(inputs["d"], dtype=np.float32))

    adjmi = adj.copy()
    np.fill_diagonal(adjmi, np.diagonal(adjmi) - 1.0)
    adjmi_b = adjmi.astype(BFNP)
    x_b = x.astype(BFNP)
    x0_b = x0.astype(BFNP)
    wT = np.ascontiguousarray(w.T.astype(BFNP))

    in_maps = []
    for c in range(N_CORES):
        r0 = c * RB
        f0 = c * FBR
        ml = np.zeros((P, NKC * 2), np.float32)
        ml[:, 2 * (2 * c)] = 1.0          # chunk 2c, half 0
        ml[:, 2 * (2 * c + 1) + 1] = 1.0  # chunk 2c+1, half 1
        mr = np.zeros((P, DKC), np.float32)
        mr[:, c] = 1.0
        in_maps.append({
            "adjmi_rows": np.ascontiguousarray(adjmi_b[r0:r0 + RB, :]),
            "alpha_blk": np.ascontiguousarray(alpha[r0:r0 + RB]),
            "x_colT": np.ascontiguousarray(x[r0:r0 + RB, :].T),
            "x0_colT": np.ascontiguousarray(x0[r0:r0 + RB, :].T),
            "w_colT": np.ascontiguousarray(w[f0:f0 + FBR, :].T),
            "masksL": ml,
            "masksR": mr,
            "adjmi_bf": adjmi_b,
            "alpha_full": alpha,
            "x_bf": x_b,
            "x0_bf": x0_b,
            "wT_full": wT,
            "d_full": d,
        })
    return in_maps


def kernel(**inputs) -> np.ndarray:
    nc = _get_nc()
    in_maps = make_in_maps(inputs)
    res = run_bass_kernel_spmd(nc, in_maps, core_ids=list(range(N_CORES)))
    z = np.concatenate([res.results[c]["z_loc"] for c in range(N_CORES)], axis=0)
    return np.ascontiguousarray(z.astype(np.float32))


if __name__ == "__main__":
    rng = np.random.default_rng(0)
    ins = {
        "x": rng.standard_normal((N, D)).astype(np.float32),
        "x0": rng.standard_normal((N, D)).astype(np.float32),
        "adj": (rng.random((N, N)) / N).astype(np.float32),
        "alpha_train": rng.standard_normal((N,)).astype(np.float32),
        "w": (np.eye(D) + 0.02 * rng.standard_normal((D, D))).astype(np.float32),
        "d": rng.random((D,)).astype(np.float32),
    }
    out = kernel(**ins)
    print("kernel output:", out.shape, out.dtype, float(np.linalg.norm(out)))
